# revision 10
# baseline (speedup 1.0000x reference)
"""Trainium2 Bass kernel for a dense transformer block (attention + ReLU FFN).

Reference computation (B=4, T=2048, C=1024, H=16, D=64):
    q,k,v = per-head projections of x;  causal softmax(q k^T / sqrt(C)) v;
    concat heads;  y = relu(out @ Wf.T + bf)

Sharding over 8 NeuronCores: core (2b+p) handles batch b with heads
[8p, 8p+8).  Attention runs causally over the full T on each core.  Pair
AllGathers (cores 2b/2b+1) share the attention outputs, and each core
runs the FFN for all 2048 tokens over its own half of the output
channels (the channel split is carried entirely by per-core input data -
every core executes an identical NEFF).

Layouts: scores are computed transposed ([s, t], keys on partitions) so
the exp() output feeds the AV matmul directly; V carries a PREPENDED
ones-column so row 0 of the AV accumulator is the softmax denominator
(landing on partition 0 where the custom-DVE reciprocal needs it);
causal masking zeroes the diagonal block's lower triangle on the DVE
after exp (a 0/1 triu multiply).  The 1/Z broadcast across partitions
is a rank-1 matmul into PSUM (a DMA broadcast costs ~11us of latency).
The FFN computes y transposed ([co, t]) so the bias+relu fuse into a
single scalar-engine activation (bias is per-partition); the host
transposes the per-core [COH, T] result back.

Engine budget: PE ~232us of streamed columns is the floor; ACT carries
exp+avc-evac+relu (~180us); DVE only does PSUM-evac casts + reciprocal;
GpSimd does the normalize multiply + broadcasts + collective triggers.
Collectives are 5 pair-AllGathers (two 512-row for th0, one 512 + two
256 for th1) emitted as early as their heads complete so only the last
(256 rows) is tail-exposed.  Compute dtype bf16 with fp32 PSUM
accumulation.
"""

import os
import sys

import numpy as np
import ml_dtypes

# Defensive: reset wedged NeuronCores on first init (must be set before the
# runtime initializes; a prior crashed process can leave cores unrecoverable)
os.environ.setdefault("NEURON_RT_RESET_CORES", "1")

for _p in ("/opt/trn_rl_repo", "/root/.axon_site/_ro/trn_rl_repo"):
    if os.path.isdir(_p) and _p not in sys.path:
        sys.path.append(_p)

B, T, C, H, D = 4, 2048, 1024, 16, 64
P = 128           # partitions
NCT = C // P      # 8 c-tiles
NTT = T // P      # 16 s/t-tiles
HPC = H // 2      # 8 heads per core
THALF = T // 2    # tokens per AllGather half
COH = C // 2      # output channels per core in the FFN
NOT = COH // P    # 4 co-tiles
SCALE = float(C) ** -0.5

bf16 = ml_dtypes.bfloat16

_CACHE = {}


def build_nc():
    import concourse.bass as bass
    import concourse.tile as tile
    from concourse import bacc, mybir

    f32 = mybir.dt.float32
    b16 = mybir.dt.bfloat16
    EXP = mybir.ActivationFunctionType.Exp
    RELU = mybir.ActivationFunctionType.Relu

    nc = bacc.Bacc("TRN2", target_bir_lowering=False, debug=False, num_devices=8)

    xT = nc.dram_tensor("xT", [C, T], b16, kind="ExternalInput").ap()
    wq = nc.dram_tensor("wq", [C, HPC * D], b16, kind="ExternalInput").ap()
    wk = nc.dram_tensor("wk", [C, HPC * D], b16, kind="ExternalInput").ap()
    wv = nc.dram_tensor("wv", [C, HPC * D], b16, kind="ExternalInput").ap()
    wfT = nc.dram_tensor("wfT", [C, COH], b16, kind="ExternalInput").ap()
    mku = nc.dram_tensor("mku", [P, P], b16, kind="ExternalInput").ap()
    biasco = nc.dram_tensor("biasco", [P, NOT], f32, kind="ExternalInput").ap()
    y = nc.dram_tensor("y", [COH, T], b16, kind="ExternalOutput").ap()

    RG = [[0, 1], [2, 3], [4, 5], [6, 7]]

    with tile.TileContext(nc) as tc, \
            tc.tile_pool(name="consts", bufs=1) as consts, \
            tc.tile_pool(name="dram", bufs=1, space="DRAM") as dram, \
            tc.tile_pool(name="sc_ps", bufs=2, space="PSUM") as sc_pool, \
            tc.tile_pool(name="av_ps", bufs=1, space="PSUM") as av_pool, \
            tc.tile_pool(name="flex_ps", bufs=2, space="PSUM") as flex_pool, \
            tc.tile_pool(name="wt", bufs=3) as wt_pool, \
            tc.tile_pool(name="norm", bufs=3) as norm_pool, \
            tc.tile_pool(name="yout", bufs=3) as y_pool:

        xT_sb = consts.tile([P, NCT, T], b16)
        wq_sb = consts.tile([P, NCT, HPC * D], b16)
        wk_sb = consts.tile([P, NCT, HPC * D], b16)
        wv_sb = consts.tile([P, NCT, HPC * D], b16)
        wfT_sb = consts.tile([P, NCT, COH], b16)
        mku_sb = consts.tile([P, P], b16)
        biasco_sb = consts.tile([P, NOT], f32)
        qT_sb = consts.tile([P, HPC // 2, T], b16)
        kT_sb = consts.tile([P, HPC // 2, T], b16)
        v_sb = consts.tile([P, NTT, HPC, 1 + D], b16)
        ccout_sb = consts.tile([P, 2, NCT, THALF], b16)
        warm_sb = consts.tile([P, 8], f32)
        ones_sb = consts.tile([1, 1 + D], b16)

        cc_in = [dram.tile([HPC * D, THALF], b16, name=f"cc_in{i}", tag=f"cc_in{i}")
                 for i in (0, 1)]

        nc.vector.memset(warm_sb, 0.0)
        nc.vector.memset(ones_sb, 1.0)
        nc.vector.memset(v_sb[:, :, :, 0:1], 1.0)

        # ---- constant loads, spread over four DMA queues so the first QK
        # projection chunks are fed within ~5us -----------------------------
        xT_r = xT.rearrange("(ct p) t -> ct p t", p=P)
        wq_r = wq.rearrange("(ct p) m -> ct p m", p=P)
        wk_r = wk.rearrange("(ct p) m -> ct p m", p=P)
        wv_r = wv.rearrange("(ct p) m -> ct p m", p=P)
        wfT_r = wfT.rearrange("(ct p) co -> ct p co", p=P)
        xT_r2 = xT.rearrange("(cp p) t -> cp p t", p=2 * P)
        wq_r2 = wq.rearrange("(cp p) m -> cp p m", p=2 * P)
        wk_r2 = wk.rearrange("(cp p) m -> cp p m", p=2 * P)
        wv_r2 = wv.rearrange("(cp p) m -> cp p m", p=2 * P)
        for ct in range(NCT):
            nc.scalar.dma_start(out=wq_sb[:, ct, :], in_=wq_r[ct])
            nc.gpsimd.dma_start(out=wk_sb[:, ct, :], in_=wk_r[ct])
            (nc.sync if ct % 2 == 0 else nc.scalar).dma_start(
                out=xT_sb[:, ct, 0:THALF], in_=xT_r[ct][:, 0:THALF])
        nc.gpsimd.dma_start(out=mku_sb, in_=mku)
        for ct in range(NCT):
            nc.gpsimd.dma_start(out=wv_sb[:, ct, :], in_=wv_r[ct])
        for ct in range(NCT):
            (nc.sync if ct % 2 == 0 else nc.gpsimd).dma_start(
                out=xT_sb[:, ct, THALF:T], in_=xT_r[ct][:, THALF:T])
        for ct in range(NCT):
            nc.sync.dma_start(out=wfT_sb[:, ct, :], in_=wfT_r[ct])
        nc.sync.dma_start(out=biasco_sb, in_=biasco)
        # ACT exp-table preload (~2.7us, after the weight DMAs so it never
        # delays them; still long before the first real exp)
        nc.scalar.activation(out=warm_sb, in_=warm_sb, func=EXP)

        # ---- emission helpers ----------------------------------------------
        def qk_chunk(hp, which, g):
          with nc.named_scope("qkproj"):
            dst, w_t = ((qT_sb, wq_sb), (kT_sb, wk_sb))[which]
            ps = flex_pool.tile([P, 512], f32, tag="flex", name=f"qkps{hp}_{which}_{g}")
            for ct in range(NCT):
                nc.tensor.matmul(
                    ps, lhsT=w_t[:, ct, hp * P:(hp + 1) * P],
                    rhs=xT_sb[:, ct, 512 * g:512 * (g + 1)],
                    start=(ct == 0), stop=(ct == NCT - 1))
            nc.vector.tensor_copy(out=dst[:, hp, 512 * g:512 * (g + 1)], in_=ps)

        def v_proj(st):
          with nc.named_scope("vproj"):
            ps = flex_pool.tile([P, 512], f32, tag="flex", name=f"vps{st}")
            for ct in range(NCT):
                nc.tensor.matmul(
                    ps, lhsT=xT_sb[:, ct, P * st:P * (st + 1)],
                    rhs=wv_sb[:, ct, :],
                    start=(ct == 0), stop=(ct == NCT - 1))
            nc.vector.tensor_copy(out=v_sb[:, st, :, 1:1 + D],
                                  in_=ps.rearrange("p (h d) -> p h d", d=D))

        pending_norm = []

        def flush_norm():
            while pending_norm:
                pending_norm.pop(0)()

        def attn_unit(h, th, mid=None):
          with nc.named_scope(f"attn{th}_{h}"):
            hp, qh = divmod(h, 2)
            base = 64 * qh
            t0 = THALF * th
            av = av_pool.tile([P, THALF], f32, tag="av", name=f"av{h}_{th}")
            jmax = 8 * th + 8
            last_j = {0: 8 * th + 3, 1: jmax - 1}
            pend = None  # (j, pieces, wt) awaiting its AV emission

            def emit_av(ent):
                j, pieces, wt = ent
                for (o, e) in pieces:
                    region = 0 if o < 512 else 1
                    nc.tensor.matmul(
                        av[0:D + 1, o:e], lhsT=v_sb[:, j, h, :], rhs=wt[:, o:e],
                        start=(j == 0), stop=(j == last_j[region]))

            for j in range(jmax):
                off = max(0, P * j - t0)
                diag = P * j >= t0
                pieces = [(off, 512), (512, 1024)] if off < 512 \
                    else [(off, 1024)]
                sc = sc_pool.tile([P, THALF], f32, tag="sc", name=f"sc{h}_{th}_{j}")
                for pi, (o, e) in enumerate(pieces):
                    nc.tensor.matmul(
                        sc[:, o:e],
                        lhsT=kT_sb[base:base + 64, hp, P * j:P * (j + 1)],
                        rhs=qT_sb[base:base + 64, hp, t0 + o:t0 + e],
                        start=True, stop=True)
                wt = wt_pool.tile([P, THALF], b16, tag="wt", name=f"wt{h}_{th}_{j}")
                nc.scalar.activation(out=wt[:, off:THALF], in_=sc[:, off:THALF],
                                     func=EXP, scale=SCALE)
                if diag:  # causal mask: zero the diagonal block's lower
                    # triangle on the DVE (cheaper than a PE mask-matmul;
                    # NOT on gpsimd - that queue carries the AllGather
                    # triggers, which must never sit behind exp-waits)
                    nc.vector.tensor_mul(out=wt[:, off:off + P],
                                         in0=wt[:, off:off + P], in1=mku_sb)
                if pend is not None:
                    emit_av(pend)
                if j == 2:
                    flush_norm()  # previous unit's zbp/stage, ~4us after its
                    # last AV so the PE never waits on the DVE recip chain
                if mid is not None and j in mid:
                    for f in mid[j]:
                        f()
                pend = (j, pieces, wt)
            emit_av(pend)
            # reciprocal straight off the PSUM denominator row (so the zbp
            # matmul unblocks ~2us earlier than recip-after-copy), then
            # evacuate the accumulator (frees the PSUM slot for the next unit)
            zr = norm_pool.tile([1, THALF], f32, tag="zr", name=f"zr{h}_{th}")
            nc.vector.reciprocal_approx_fast(out=zr, in_=av[0:1, 0:THALF])
            zrb = norm_pool.tile([1, THALF], b16, tag="zrb", name=f"zrb{h}_{th}")
            nc.vector.tensor_copy(out=zrb, in_=zr)
            avc = norm_pool.tile([D + 1, THALF], f32, tag="avc", name=f"avc{h}_{th}")
            nc.vector.tensor_copy(out=avc, in_=av[0:D + 1, 0:THALF])

            def norm_tail(h=h, th=th, avc=avc, zrb=zrb):
                # broadcast 1/Z across partitions with a rank-1 matmul (a
                # DMA broadcast costs ~11us of latency; the PE does it in
                # 0.4us); deferred into the next unit so the PE queue never
                # stalls on the DVE recip chain
                zbp = sc_pool.tile([P, THALF], f32, tag="sc", name=f"zbp{h}_{th}")
                for o in (0, 512):  # one MM per PSUM bank
                    nc.tensor.matmul(zbp[0:1 + D, o:o + 512], lhsT=ones_sb,
                                     rhs=zrb[:, o:o + 512], start=True, stop=True)
                # row 0 of stage is Z/Z (garbage); the cc_in DMA skips it
                # (DMAs have no partition-base alignment constraint)
                stage = norm_pool.tile([1 + D, THALF], b16, tag="stage",
                                       name=f"st{h}_{th}")
                nc.vector.tensor_mul(out=stage, in0=avc, in1=zbp[0:1 + D, :])
                # the last unit's stage-out rides the gpsimd queue so it is
                # not stuck behind tail ccout loads on sync, and the AG
                # trigger directly follows it in its own queue
                q = nc.gpsimd if (h, th) == (7, 1) else nc.sync
                q.dma_start(out=cc_in[th][64 * h:64 * (h + 1), :],
                            in_=stage[1:1 + D, :])

            pending_norm.append(norm_tail)

        ag_state = {}

        def allgather(th, r0, r1, nm):
          # trigger only; the SBUF loads are emitted later (ccout_load) so
          # their CC-completion waits never sit ahead of later stage-out
          # DMAs in the sync queue
          with nc.named_scope(f"ag_{nm}"):
            out_t = dram.tile([2 * (r1 - r0), THALF], b16,
                              name=f"ag_{nm}", tag=f"ag_{nm}")
            nc.gpsimd.collective_compute(
                "AllGather", mybir.AluOpType.bypass, replica_groups=RG,
                ins=[cc_in[th][r0:r1, :].opt()], outs=[out_t.opt()])
            ag_state[nm] = out_t

        def ccout_load(th, cis, nm):
            out_r = ag_state[nm].rearrange("(k p) t -> k p t", p=P)
            for k, ci in enumerate(cis):
                nc.sync.dma_start(out=ccout_sb[:, th, ci, :], in_=out_r[k])

        y_r = y.rearrange("(ot p) t -> ot p t", p=P)

        FFN_CI_ORDER = (0, 1, 4, 5, 2, 6, 3, 7)  # AllGather arrival order

        def ffn_mms(ps, th, ot, tc, cis, start, stop):
            for k, ci in enumerate(cis):
                nc.tensor.matmul(
                    ps, lhsT=wfT_sb[:, ci, P * ot:P * (ot + 1)],
                    rhs=ccout_sb[:, th, ci, 512 * tc:512 * (tc + 1)],
                    start=(start and k == 0), stop=(stop and k == len(cis) - 1))

        def ffn_out(ps, th, ot, tc):
            ysb = y_pool.tile([P, 512], b16, tag="y", name=f"y{th}_{ot}_{tc}")
            nc.scalar.activation(out=ysb, in_=ps, func=RELU,
                                 bias=biasco_sb[:, ot:ot + 1])
            t0 = THALF * th
            nc.sync.dma_start(out=y_r[ot][:, t0 + 512 * tc:t0 + 512 * (tc + 1)],
                              in_=ysb)

        def ffn_tile(th, ot, tc):
          with nc.named_scope("ffn"):
            ps = flex_pool.tile([P, 512], f32, tag="flex", name=f"fps{th}_{ot}_{tc}")
            ffn_mms(ps, th, ot, tc, FFN_CI_ORDER, True, True)
            ffn_out(ps, th, ot, tc)

        # ---- emission order -------------------------------------------------
        def qk4(hp, gs):
            return [lambda w=w, g=g, hp=hp: qk_chunk(hp, w, g)
                    for g in gs for w in (0, 1)]

        def vshots(sts):
            return [lambda st=st: v_proj(st) for st in sts]

        def fshots(specs):
            return [lambda s=s: ffn_tile(*s) for s in specs]

        def mids(fs, js):
            return {j: [f] for j, f in zip(js, fs)}

        # startup: q/k for head-pair 0 over the th0 token columns + first v
        for f in qk4(0, (0, 1)) + vshots(range(4)):
            f()

        # th0 attention; projection chunks fill unit boundaries
        attn_unit(0, 0, mid=mids(vshots(range(4, 8)), (0, 1, 2, 3)))
        for f in qk4(1, (0, 1)):
            f()
        attn_unit(1, 0)
        for f in qk4(2, (0, 1)):
            f()
        attn_unit(2, 0)
        for f in qk4(3, (0, 1)):
            f()
        attn_unit(3, 0)
        for f in qk4(0, (2,)):
            f()
        attn_unit(4, 0, mid={3: [lambda: allgather(0, 0, 256, "th0a")]})
        for f in qk4(0, (3,)):
            f()
        attn_unit(5, 0)
        ccout_load(0, (0, 1, 4, 5), "th0a")
        v_proj(8)
        attn_unit(6, 0)
        v_proj(9)
        v_proj(10)
        attn_unit(7, 0)
        v_proj(11)

        # th1 attention; q/k th1 columns + th0 FFN tiles fill the units
        attn_unit(0, 1, mid=dict(
            list(mids(vshots(range(12, 16)) + qk4(1, (2, 3)),
                      (0, 1, 4, 5, 7, 9, 11, 13)).items())
            + [(3, [lambda: allgather(0, 256, 512, "th0b")])]))
        ccout_load(0, (2, 3, 6, 7), "th0b")
        attn_unit(1, 1, mid=mids(qk4(2, (2, 3)), (1, 4, 6, 8)))
        attn_unit(2, 1, mid=mids(fshots([(0, 0, 0)]), (6,)))
        attn_unit(3, 1, mid=mids(fshots([(0, 0, 1)]), (6,)))
        attn_unit(4, 1, mid=dict(
            list(mids(fshots([(0, 1, 0)]) + qk4(3, (2,)), (6, 9, 12)).items())
            + [(3, [lambda: allgather(1, 0, 256, "th1a")])]))
        attn_unit(5, 1, mid=mids(fshots([(0, 1, 1)]) + qk4(3, (3,)), (4, 8, 11)))
        attn_unit(6, 1, mid=dict(
            list(mids(fshots([(0, 2, 0), (0, 2, 1)]), (6, 10)).items())
            + [(3, [lambda: allgather(1, 256, 384, "th1b")])]))
        ccout_load(1, (0, 1, 4, 5), "th1a")
        attn_unit(7, 1, mid=dict(
            list(mids(fshots([(0, 3, 0), (0, 3, 1)]), (6, 11)).items())
            + [(3, [lambda: allgather(1, 384, 448, "th1c")]),  # head 6
               (8, [lambda: ccout_load(1, (2, 6), "th1b")])]))

        # ---- tail: park partial FFN accumulations for all 8 th1 tiles in
        # the now-idle attention PSUM.  partA carries only (0,1,4,5); the
        # (2,6) fulls plus the th1c-halves (K=64, head 6 / peer head 14) of
        # ci3/ci7 then fill the last AllGather's latency, keeping the PE
        # warm; after th1d lands only the K=64 bottom halves (head 7 / peer
        # head 15) and the relu+store remain.
        CIS_A = (0, 1, 4, 5)
        hosts = []

        def host_partA(ot, pool, tag):
            if pool is flex_pool:
                h0 = pool.tile([P, 512], f32, tag=tag, name=f"tf{ot}a")
                h1 = pool.tile([P, 512], f32, tag=tag, name=f"tf{ot}b")
                pair = ((h0, 0), (h1, 0))
            else:
                ht = pool.tile([P, THALF], f32, tag=tag, name=f"tf{ot}")
                pair = ((ht, 0), (ht, 512))
            for tc, (ht, c0) in enumerate(pair):
                ffn_mms(ht[:, c0:c0 + 512], 1, ot, tc, CIS_A, True, False)
                hosts.append((ot, tc, ht, c0))

        def ffn_half(ht, lo, hi, ot, tc, cis, stop):
            for k, ci in enumerate(cis):
                nc.tensor.matmul(
                    ht, lhsT=wfT_sb[lo:hi, ci, P * ot:P * (ot + 1)],
                    rhs=ccout_sb[lo:hi, 1, ci, 512 * tc:512 * (tc + 1)],
                    start=False, stop=(stop and k == len(cis) - 1))

        # flex hosts first (PE work while the DVE recip chain of unit (7,1)
        # completes), then the norm tail (zbp must take its sc slot BEFORE
        # the sc hosts, else the slot rotation deadlocks), then sc/av hosts
        host_partA(0, flex_pool, "flex")
        flush_norm()
        host_partA(1, sc_pool, "sc")
        host_partA(2, sc_pool, "sc")
        host_partA(3, av_pool, "av")
        # head 7 of th1: the only tail-exposed collective (128KB)
        allgather(1, 448, 512, "th1d")
        out_c = ag_state["th1c"].rearrange("(k p) t -> k p t", p=64)
        nc.sync.dma_start(out=ccout_sb[0:64, 1, 3, :], in_=out_c[0])
        nc.sync.dma_start(out=ccout_sb[0:64, 1, 7, :], in_=out_c[1])
        # AllGather-latency fill: (2,6) fulls, then ci3/ci7 top halves
        for ot, tc, ht, c0 in hosts:
            ffn_mms(ht[:, c0:c0 + 512], 1, ot, tc, (2, 6), False, False)
        for ot, tc, ht, c0 in hosts:
            ffn_half(ht[:, c0:c0 + 512], 0, 64, ot, tc, (3, 7), False)
        out_d = ag_state["th1d"].rearrange("(k p) t -> k p t", p=64)
        nc.scalar.dma_start(out=ccout_sb[64:P, 1, 3, :], in_=out_d[0])
        nc.scalar.dma_start(out=ccout_sb[64:P, 1, 7, :], in_=out_d[1])
        for ot, tc, ht, c0 in hosts:
            ffn_half(ht[:, c0:c0 + 512], 64, P, ot, tc, (3, 7), True)
            ffn_out(ht[:, c0:c0 + 512], 1, ot, tc)

    nc.compile()
    return nc


def make_in_maps(x, Wq, Wk, Wv, Wf, bf):
    x = np.asarray(x, np.float32)
    mku_m = np.ascontiguousarray(
        np.triu(np.ones((P, P), np.float32))).astype(bf16)
    bf_f = np.asarray(bf, np.float32)
    wfT_f = np.asarray(Wf, np.float32).T
    in_maps = []
    for core in range(8):
        b, p = divmod(core, 2)
        sl = slice(HPC * p, HPC * (p + 1))
        in_maps.append({
            "xT": np.ascontiguousarray(x[b].T).astype(bf16),
            "wq": np.ascontiguousarray(
                np.asarray(Wq, np.float32)[:, sl].reshape(C, HPC * D)).astype(bf16),
            "wk": np.ascontiguousarray(
                np.asarray(Wk, np.float32)[:, sl].reshape(C, HPC * D)).astype(bf16),
            "wv": np.ascontiguousarray(
                np.asarray(Wv, np.float32)[:, sl].reshape(C, HPC * D)).astype(bf16),
            "wfT": np.ascontiguousarray(
                wfT_f[:, COH * p:COH * (p + 1)]).astype(bf16),
            "mku": mku_m,
            "biasco": np.ascontiguousarray(
                bf_f[COH * p:COH * (p + 1)].reshape(NOT, P).T),
        })
    return in_maps


def run(x, Wq, Wk, Wv, Wf, bf, trace=False, **spmd_kwargs):
    from concourse.bass_utils import run_bass_kernel_spmd

    if "nc" not in _CACHE:
        _CACHE["nc"] = build_nc()
    nc = _CACHE["nc"]
    in_maps = make_in_maps(x, Wq, Wk, Wv, Wf, bf)
    res = run_bass_kernel_spmd(
        nc, in_maps, core_ids=list(range(8)), trace=trace, **spmd_kwargs)
    out = np.zeros((B, T, C), np.float32)
    for core in range(8):
        b, p = divmod(core, 2)
        out[b, :, COH * p:COH * (p + 1)] = \
            res.results[core]["y"].T.astype(np.float32)
    return out, res


def kernel(x, Wq, Wk, Wv, Wf, bf):
    try:
        out, _ = run(x, Wq, Wk, Wv, Wf, bf, trace=False)
    except Exception:
        # transient device flake (hang/unrecoverable): one retry
        import time
        time.sleep(2.0)
        out, _ = run(x, Wq, Wk, Wv, Wf, bf, trace=False)
    return out



# revision 30
# speedup vs baseline: 1.0568x; 1.0568x over previous
"""Trainium2 Bass kernel for a dense transformer block (attention + ReLU FFN).

Reference computation (B=4, T=2048, C=1024, H=16, D=64):
    q,k,v = per-head projections of x;  causal softmax(q k^T / sqrt(C)) v;
    concat heads;  y = relu(out @ Wf.T + bf)

Sharding over 8 NeuronCores: core (2b+p) handles batch b with heads
[8p, 8p+8).  Attention runs causally over the full T on each core.  Pair
AllGathers (cores 2b/2b+1) share the attention outputs, and each core
runs the FFN for all 2048 tokens over its own half of the output
channels (the channel split is carried entirely by per-core input data -
every core executes an identical NEFF).

Layouts: scores are computed transposed ([s, t], keys on partitions) so
the exp() output feeds the AV matmul directly; V carries a PREPENDED
ones-column so row 0 of the AV accumulator is the softmax denominator
(landing on partition 0 where the custom-DVE reciprocal needs it);
causal masking zeroes the diagonal block's lower triangle on the DVE
after exp (a 0/1 triu multiply).  The 1/Z broadcast across partitions
is a rank-1 matmul into PSUM (a DMA broadcast costs ~11us of latency).
The FFN computes y transposed ([co, t]) so the bias+relu fuse into a
single scalar-engine activation (bias is per-partition); the host
transposes the per-core [COH, T] result back.

Engine budget: PE ~232us of streamed columns is the floor; ACT carries
exp+avc-evac+relu (~180us); DVE only does PSUM-evac casts + reciprocal;
GpSimd does the normalize multiply + broadcasts + collective triggers.
Collectives are 5 pair-AllGathers (two 512-row for th0, one 512 + two
256 for th1) emitted as early as their heads complete so only the last
(256 rows) is tail-exposed.  Compute dtype bf16 with fp32 PSUM
accumulation.
"""

import os
import sys

import numpy as np
import ml_dtypes

# Defensive: reset wedged NeuronCores on first init (must be set before the
# runtime initializes; a prior crashed process can leave cores unrecoverable)
os.environ.setdefault("NEURON_RT_RESET_CORES", "1")

for _p in ("/opt/trn_rl_repo", "/root/.axon_site/_ro/trn_rl_repo"):
    if os.path.isdir(_p) and _p not in sys.path:
        sys.path.append(_p)

B, T, C, H, D = 4, 2048, 1024, 16, 64
P = 128           # partitions
NCT = C // P      # 8 c-tiles
NTT = T // P      # 16 s/t-tiles
HPC = H // 2      # 8 heads per core
THALF = T // 2    # tokens per AllGather half
COH = C // 2      # output channels per core in the FFN
NOT = COH // P    # 4 co-tiles
SCALE = float(C) ** -0.5

# fp8 DoubleRow for the QKV projections: x and Wq/Wk/Wv ship as fp8e4m3
# (host pre-scales the weights by W_SCALE to clear the subnormal zone; the
# q/k scale cancels in the exp argument, the v scale cancels against the
# denominator because the ones-column is also W_SCALE).  Halves both the
# projection matmul streaming time and the input HBM traffic.
FP8_QKV = False
W_SCALE = 16.0
# q and k each carry a factor of W_SCALE under FP8_QKV; fold it out of the
# score scaling inside the exp's free affine
EXP_SCALE = SCALE / (W_SCALE * W_SCALE) if FP8_QKV else SCALE

# fp8 DoubleRow for the AV matmul: the exp writes fp8 weights for key-block
# PAIRS ([P, 2, THALF] tiles) and one DoubleRow matmul contracts both blocks
# (virtual K=256), halving the AV streaming time.  Causal masking moves
# PRE-exp: an additive -BIG mask on the fp32 scores (the DVE then never
# touches fp8), so exp emits exact zeros below the diagonal.
FP8_AV = False
MASK_BIG = 1.0e6

bf16 = ml_dtypes.bfloat16
f8e4 = ml_dtypes.float8_e4m3

_CACHE = {}


def build_nc():
    import concourse.bass as bass
    import concourse.tile as tile
    from concourse import bacc, mybir

    f32 = mybir.dt.float32
    b16 = mybir.dt.bfloat16
    f8 = mybir.dt.float8e4
    xdt = f8 if FP8_QKV else b16
    DR = mybir.MatmulPerfMode.DoubleRow
    EXP = mybir.ActivationFunctionType.Exp
    RELU = mybir.ActivationFunctionType.Relu

    nc = bacc.Bacc("TRN2", target_bir_lowering=False, debug=False, num_devices=8)

    xT = nc.dram_tensor("xT", [C, T], xdt, kind="ExternalInput").ap()
    wq = nc.dram_tensor("wq", [C, HPC * D], xdt, kind="ExternalInput").ap()
    wk = nc.dram_tensor("wk", [C, HPC * D], xdt, kind="ExternalInput").ap()
    wv = nc.dram_tensor("wv", [C, HPC * D], xdt, kind="ExternalInput").ap()
    wfT = nc.dram_tensor("wfT", [C, COH], b16, kind="ExternalInput").ap()
    mku = nc.dram_tensor("mku", [P, P], b16, kind="ExternalInput").ap()
    mkb = nc.dram_tensor("mkb", [P, 2 * P], b16, kind="ExternalInput").ap()
    biasco = nc.dram_tensor("biasco", [P, NOT], f32, kind="ExternalInput").ap()
    y = nc.dram_tensor("y", [COH, T], b16, kind="ExternalOutput").ap()

    RG = [[0, 1], [2, 3], [4, 5], [6, 7]]

    with tile.TileContext(nc) as tc, \
            tc.tile_pool(name="consts", bufs=1) as consts, \
            tc.tile_pool(name="dram", bufs=1, space="DRAM") as dram, \
            tc.tile_pool(name="sc_ps", bufs=2, space="PSUM") as sc_pool, \
            tc.tile_pool(name="av_ps", bufs=1, space="PSUM") as av_pool, \
            tc.tile_pool(name="flex_ps", bufs=2, space="PSUM") as flex_pool, \
            tc.tile_pool(name="wt", bufs=3) as wt_pool, \
            tc.tile_pool(name="norm", bufs=3) as norm_pool, \
            tc.tile_pool(name="yout", bufs=3) as y_pool:

        xT_sb = consts.tile([P, NCT, T], xdt)
        wq_sb = consts.tile([P, NCT, HPC * D], xdt)
        wk_sb = consts.tile([P, NCT, HPC * D], xdt)
        wv_sb = consts.tile([P, NCT, HPC * D], xdt)
        wfT_sb = consts.tile([P, NCT, COH], b16)
        mku_sb = consts.tile([P, P], b16)
        mkb_sb = consts.tile([P, 2 * P], b16)
        biasco_sb = consts.tile([P, NOT], f32)
        qT_sb = consts.tile([P, HPC // 2, T], b16)
        kT_sb = consts.tile([P, HPC // 2, T], b16)
        # fp8 AV: 66-wide v rows (ones-col + 64 data + a zero pad column so
        # the DoubleRow k-pair stride is 16B-aligned); bf16 AV: 65-wide
        if FP8_AV:
            v_sb = consts.tile([P, NTT, HPC, 2 + D], f8, name="v_sb")
        else:
            v_sb = consts.tile([P, NTT, HPC, 1 + D], b16, name="v_sb")
        ccout_sb = consts.tile([P, 2, NCT, THALF], b16)
        warm_sb = consts.tile([P, 8], f32)
        ones_sb = consts.tile([1, 1 + D], b16)

        cc_in = [dram.tile([HPC * D, THALF], b16, name=f"cc_in{i}", tag=f"cc_in{i}")
                 for i in (0, 1)]

        nc.vector.memset(warm_sb, 0.0)
        nc.vector.memset(ones_sb, 1.0)
        # the ones-column must carry the same scale as the fp8-projected v
        # rows so the softmax denominator (AV row 0) divides it out exactly
        nc.vector.memset(v_sb[:, :, :, 0:1], W_SCALE if FP8_QKV else 1.0)
        if FP8_AV:
            nc.vector.memset(v_sb[:, :, :, 1 + D:2 + D], 0.0)
            # zero the two score-PSUM ring slots once: the paired exp reads
            # masked-stale columns there, which must be finite (not NaN/Inf)
            for i in (0, 1):
                z = sc_pool.tile([P, THALF], f32, tag="sc", name=f"scz{i}")
                nc.vector.memset(z, 0.0)

        # ---- constant loads, spread over four DMA queues so the first QK
        # projection chunks are fed within ~5us -----------------------------
        xT_r = xT.rearrange("(ct p) t -> ct p t", p=P)
        wq_r = wq.rearrange("(ct p) m -> ct p m", p=P)
        wk_r = wk.rearrange("(ct p) m -> ct p m", p=P)
        wv_r = wv.rearrange("(ct p) m -> ct p m", p=P)
        wfT_r = wfT.rearrange("(ct p) co -> ct p co", p=P)
        xT_r2 = xT.rearrange("(cp p) t -> cp p t", p=2 * P)
        wq_r2 = wq.rearrange("(cp p) m -> cp p m", p=2 * P)
        wk_r2 = wk.rearrange("(cp p) m -> cp p m", p=2 * P)
        wv_r2 = wv.rearrange("(cp p) m -> cp p m", p=2 * P)
        for ct in range(NCT):
            nc.scalar.dma_start(out=wq_sb[:, ct, :], in_=wq_r[ct])
            nc.gpsimd.dma_start(out=wk_sb[:, ct, :], in_=wk_r[ct])
            (nc.sync if ct % 2 == 0 else nc.scalar).dma_start(
                out=xT_sb[:, ct, 0:THALF], in_=xT_r[ct][:, 0:THALF])
        nc.gpsimd.dma_start(out=mku_sb, in_=mku)
        nc.gpsimd.dma_start(out=mkb_sb, in_=mkb)
        for ct in range(NCT):
            nc.gpsimd.dma_start(out=wv_sb[:, ct, :], in_=wv_r[ct])
        for ct in range(NCT):
            (nc.sync if ct % 2 == 0 else nc.gpsimd).dma_start(
                out=xT_sb[:, ct, THALF:T], in_=xT_r[ct][:, THALF:T])
        for ct in range(NCT):
            nc.sync.dma_start(out=wfT_sb[:, ct, :], in_=wfT_r[ct])
        nc.sync.dma_start(out=biasco_sb, in_=biasco)
        # ACT exp-table preload (~2.7us, after the weight DMAs so it never
        # delays them; still long before the first real exp)
        nc.scalar.activation(out=warm_sb, in_=warm_sb, func=EXP)

        # ---- emission helpers ----------------------------------------------
        def qk_chunk(hp, which, g):
          with nc.named_scope("qkproj"):
            dst, w_t = ((qT_sb, wq_sb), (kT_sb, wk_sb))[which]
            ps = flex_pool.tile([P, 512], f32, tag="flex", name=f"qkps{hp}_{which}_{g}")
            if FP8_QKV:
                for c in range(NCT // 2):
                    nc.tensor.matmul(
                        ps, lhsT=w_t[:, 2 * c:2 * c + 2, hp * P:(hp + 1) * P],
                        rhs=xT_sb[:, 2 * c:2 * c + 2, 512 * g:512 * (g + 1)],
                        start=(c == 0), stop=(c == NCT // 2 - 1), perf_mode=DR)
            else:
                for ct in range(NCT):
                    nc.tensor.matmul(
                        ps, lhsT=w_t[:, ct, hp * P:(hp + 1) * P],
                        rhs=xT_sb[:, ct, 512 * g:512 * (g + 1)],
                        start=(ct == 0), stop=(ct == NCT - 1))
            nc.vector.tensor_copy(out=dst[:, hp, 512 * g:512 * (g + 1)], in_=ps)

        def v_proj(st):
          with nc.named_scope("vproj"):
            ps = flex_pool.tile([P, 512], f32, tag="flex", name=f"vps{st}")
            if FP8_QKV:
                for c in range(NCT // 2):
                    nc.tensor.matmul(
                        ps, lhsT=xT_sb[:, 2 * c:2 * c + 2, P * st:P * (st + 1)],
                        rhs=wv_sb[:, 2 * c:2 * c + 2, :],
                        start=(c == 0), stop=(c == NCT // 2 - 1), perf_mode=DR)
            else:
                for ct in range(NCT):
                    nc.tensor.matmul(
                        ps, lhsT=xT_sb[:, ct, P * st:P * (st + 1)],
                        rhs=wv_sb[:, ct, :],
                        start=(ct == 0), stop=(ct == NCT - 1))
            nc.vector.tensor_copy(out=v_sb[:, st, :, 1:1 + D],
                                  in_=ps.rearrange("p (h d) -> p h d", d=D))

        pending_norm = []

        def flush_norm():
            while pending_norm:
                pending_norm.pop(0)()

        def attn_unit(h, th, mid=None):
          with nc.named_scope(f"attn{th}_{h}"):
            hp, qh = divmod(h, 2)
            base = 64 * qh
            t0 = THALF * th
            av = av_pool.tile([P, THALF], f32, tag="av", name=f"av{h}_{th}")
            jmax = 8 * th + 8
            last_j = {0: 8 * th + 3, 1: jmax - 1}
            last_m = {0: 4 * th + 1, 1: 4 * th + 3}
            pend = None  # weights awaiting their AV emission

            def emit_av(ent):
                if FP8_AV:
                    m, offm, wtp = ent
                    pieces = [(offm, 512), (512, 1024)] if offm < 512 \
                        else [(offm, 1024)]
                    for (o, e) in pieces:
                        region = 0 if o < 512 else 1
                        nc.tensor.matmul(
                            av[0:D + 2, o:e],
                            lhsT=v_sb[:, 2 * m:2 * m + 2, h, :],
                            rhs=wtp[:, :, o:e],
                            start=(m == 0), stop=(m == last_m[region]),
                            perf_mode=DR)
                else:
                    j, pieces, wt = ent
                    for (o, e) in pieces:
                        region = 0 if o < 512 else 1
                        nc.tensor.matmul(
                            av[0:D + 1, o:e], lhsT=v_sb[:, j, h, :],
                            rhs=wt[:, o:e],
                            start=(j == 0), stop=(j == last_j[region]))

            wtp = None
            for j in range(jmax):
                m, jj = divmod(j, 2)
                off = max(0, P * j - t0)
                offm = max(0, P * (j - jj) - t0)  # pair-base column
                diag = P * j >= t0
                pieces = [(off, 512), (512, 1024)] if off < 512 \
                    else [(off, 1024)]
                sc = sc_pool.tile([P, THALF], f32, tag="sc", name=f"sc{h}_{th}_{j}")
                for pi, (o, e) in enumerate(pieces):
                    nc.tensor.matmul(
                        sc[:, o:e],
                        lhsT=kT_sb[base:base + 64, hp, P * j:P * (j + 1)],
                        rhs=qT_sb[base:base + 64, hp, t0 + o:t0 + e],
                        start=True, stop=True)
                if FP8_AV:
                    # additive -BIG causal mask on the fp32 scores (pre-exp);
                    # the odd half also wipes [off-P, off) so the paired AV
                    # sees exact zeros for its not-yet-causal key block
                    if jj == 0:
                        wtp = wt_pool.tile([P, 2, THALF], f8, tag="wt",
                                           name=f"wt{h}_{th}_{m}")
                    if diag:
                        if jj == 0:
                            nc.vector.tensor_add(
                                out=sc[:, off:off + P],
                                in0=sc[:, off:off + P], in1=mkb_sb[:, P:2 * P])
                        else:
                            nc.vector.tensor_add(
                                out=sc[:, off - P:off + P],
                                in0=sc[:, off - P:off + P], in1=mkb_sb)
                    nc.scalar.activation(out=wtp[:, jj, offm:THALF],
                                         in_=sc[:, offm:THALF],
                                         func=EXP, scale=EXP_SCALE)
                    if pend is not None and jj == 0:
                        emit_av(pend)
                else:
                    wt = wt_pool.tile([P, THALF], b16, tag="wt",
                                      name=f"wt{h}_{th}_{j}")
                    nc.scalar.activation(out=wt[:, off:THALF],
                                         in_=sc[:, off:THALF],
                                         func=EXP, scale=EXP_SCALE)
                    if diag:  # causal mask: zero the diagonal block's lower
                        # triangle on the DVE (cheaper than a PE mask-matmul;
                        # NOT on gpsimd - that queue carries the AllGather
                        # triggers, which must never sit behind exp-waits)
                        nc.vector.tensor_mul(out=wt[:, off:off + P],
                                             in0=wt[:, off:off + P], in1=mku_sb)
                    if pend is not None:
                        emit_av(pend)
                if j == 2:
                    flush_norm()  # previous unit's zbp/stage, ~4us after its
                    # last AV so the PE never waits on the DVE recip chain
                if mid is not None and j in mid:
                    for f in mid[j]:
                        f()
                if FP8_AV:
                    if jj == 1:
                        pend = (m, offm, wtp)
                else:
                    pend = (j, pieces, wt)
            emit_av(pend)
            # mid-kernel units: evacuate the accumulator FIRST (the copy is
            # what frees the av PSUM slot the next unit's AV matmuls need;
            # av_pool has bufs=1).  Last unit: reciprocal first, straight
            # off the PSUM denominator row, so the tail zbp/stage/AllGather
            # chain unblocks ~2us earlier (no next unit to starve).
            avc = norm_pool.tile([D + 1, THALF], f32, tag="avc", name=f"avc{h}_{th}")
            zr = norm_pool.tile([1, THALF], f32, tag="zr", name=f"zr{h}_{th}")
            zrb = norm_pool.tile([1, THALF], b16, tag="zrb", name=f"zrb{h}_{th}")
            if (h, th) == (7, 1):
                nc.vector.reciprocal_approx_fast(out=zr, in_=av[0:1, 0:THALF])
                nc.vector.tensor_copy(out=zrb, in_=zr)
                nc.vector.tensor_copy(out=avc, in_=av[0:D + 1, 0:THALF])
            else:
                nc.vector.tensor_copy(out=avc, in_=av[0:D + 1, 0:THALF])
                nc.vector.reciprocal_approx_fast(out=zr, in_=avc[0:1, :])
                nc.vector.tensor_copy(out=zrb, in_=zr)

            def norm_tail(h=h, th=th, avc=avc, zrb=zrb):
                # broadcast 1/Z across partitions with a rank-1 matmul (a
                # DMA broadcast costs ~11us of latency; the PE does it in
                # 0.4us); deferred into the next unit so the PE queue never
                # stalls on the DVE recip chain
                zbp = sc_pool.tile([P, THALF], f32, tag="sc", name=f"zbp{h}_{th}")
                for o in (0, 512):  # one MM per PSUM bank
                    nc.tensor.matmul(zbp[0:1 + D, o:o + 512], lhsT=ones_sb,
                                     rhs=zrb[:, o:o + 512], start=True, stop=True)
                # row 0 of stage is Z/Z (garbage); the cc_in DMA skips it
                # (DMAs have no partition-base alignment constraint)
                stage = norm_pool.tile([1 + D, THALF], b16, tag="stage",
                                       name=f"st{h}_{th}")
                nc.vector.tensor_mul(out=stage, in0=avc, in1=zbp[0:1 + D, :])
                # the last unit's stage-out rides the gpsimd queue so it is
                # not stuck behind tail ccout loads on sync, and the AG
                # trigger directly follows it in its own queue
                q = nc.gpsimd if (h, th) == (7, 1) else nc.sync
                q.dma_start(out=cc_in[th][64 * h:64 * (h + 1), :],
                            in_=stage[1:1 + D, :])

            pending_norm.append(norm_tail)

        ag_state = {}

        def allgather(th, r0, r1, nm):
          # trigger only; the SBUF loads are emitted later (ccout_load) so
          # their CC-completion waits never sit ahead of later stage-out
          # DMAs in the sync queue
          with nc.named_scope(f"ag_{nm}"):
            out_t = dram.tile([2 * (r1 - r0), THALF], b16,
                              name=f"ag_{nm}", tag=f"ag_{nm}")
            nc.gpsimd.collective_compute(
                "AllGather", mybir.AluOpType.bypass, replica_groups=RG,
                ins=[cc_in[th][r0:r1, :].opt()], outs=[out_t.opt()])
            ag_state[nm] = out_t

        def ccout_load(th, cis, nm):
            out_r = ag_state[nm].rearrange("(k p) t -> k p t", p=P)
            for k, ci in enumerate(cis):
                nc.sync.dma_start(out=ccout_sb[:, th, ci, :], in_=out_r[k])

        y_r = y.rearrange("(ot p) t -> ot p t", p=P)

        FFN_CI_ORDER = (0, 1, 4, 5, 2, 6, 3, 7)  # AllGather arrival order

        def ffn_mms(ps, th, ot, tc, cis, start, stop):
            for k, ci in enumerate(cis):
                nc.tensor.matmul(
                    ps, lhsT=wfT_sb[:, ci, P * ot:P * (ot + 1)],
                    rhs=ccout_sb[:, th, ci, 512 * tc:512 * (tc + 1)],
                    start=(start and k == 0), stop=(stop and k == len(cis) - 1))

        def ffn_out(ps, th, ot, tc):
            ysb = y_pool.tile([P, 512], b16, tag="y", name=f"y{th}_{ot}_{tc}")
            nc.scalar.activation(out=ysb, in_=ps, func=RELU,
                                 bias=biasco_sb[:, ot:ot + 1])
            t0 = THALF * th
            nc.sync.dma_start(out=y_r[ot][:, t0 + 512 * tc:t0 + 512 * (tc + 1)],
                              in_=ysb)

        def ffn_tile(th, ot, tc):
          with nc.named_scope("ffn"):
            ps = flex_pool.tile([P, 512], f32, tag="flex", name=f"fps{th}_{ot}_{tc}")
            ffn_mms(ps, th, ot, tc, FFN_CI_ORDER, True, True)
            ffn_out(ps, th, ot, tc)

        # ---- emission order -------------------------------------------------
        def qk4(hp, gs):
            return [lambda w=w, g=g, hp=hp: qk_chunk(hp, w, g)
                    for g in gs for w in (0, 1)]

        def vshots(sts):
            return [lambda st=st: v_proj(st) for st in sts]

        def fshots(specs):
            return [lambda s=s: ffn_tile(*s) for s in specs]

        def mids(fs, js):
            return {j: [f] for j, f in zip(js, fs)}

        # startup: q/k for head-pair 0 over the th0 token columns + first v
        for f in qk4(0, (0, 1)) + vshots(range(4)):
            f()

        # th0 attention; projection chunks fill unit boundaries
        attn_unit(0, 0, mid=mids(vshots(range(4, 8)), (0, 1, 2, 3)))
        for f in qk4(1, (0, 1)):
            f()
        attn_unit(1, 0)
        for f in qk4(2, (0, 1)):
            f()
        attn_unit(2, 0)
        for f in qk4(3, (0, 1)):
            f()
        attn_unit(3, 0)
        for f in qk4(0, (2,)):
            f()
        attn_unit(4, 0, mid={3: [lambda: allgather(0, 0, 256, "th0a")]})
        for f in qk4(0, (3,)):
            f()
        attn_unit(5, 0)
        ccout_load(0, (0, 1, 4, 5), "th0a")
        v_proj(8)
        attn_unit(6, 0)
        v_proj(9)
        v_proj(10)
        attn_unit(7, 0)
        v_proj(11)

        # th1 attention; q/k th1 columns + th0 FFN tiles fill the units
        attn_unit(0, 1, mid=dict(
            list(mids(vshots(range(12, 16)) + qk4(1, (2, 3)),
                      (0, 1, 4, 5, 7, 9, 11, 13)).items())
            + [(3, [lambda: allgather(0, 256, 512, "th0b")])]))
        ccout_load(0, (2, 3, 6, 7), "th0b")
        attn_unit(1, 1, mid=mids(qk4(2, (2, 3)), (1, 4, 6, 8)))
        attn_unit(2, 1, mid=mids(fshots([(0, 0, 0)]), (6,)))
        attn_unit(3, 1, mid=mids(fshots([(0, 0, 1)]), (6,)))
        attn_unit(4, 1, mid=dict(
            list(mids(fshots([(0, 1, 0)]) + qk4(3, (2,)), (6, 9, 12)).items())
            + [(3, [lambda: allgather(1, 0, 256, "th1a")])]))
        attn_unit(5, 1, mid=mids(fshots([(0, 1, 1)]) + qk4(3, (3,)), (4, 8, 11)))
        attn_unit(6, 1, mid=dict(
            list(mids(fshots([(0, 2, 0), (0, 2, 1)]), (6, 10)).items())
            + [(3, [lambda: allgather(1, 256, 384, "th1b")])]))
        ccout_load(1, (0, 1, 4, 5), "th1a")
        attn_unit(7, 1, mid=dict(
            list(mids(fshots([(0, 3, 0), (0, 3, 1)]), (6, 11)).items())
            + [(3, [lambda: allgather(1, 384, 448, "th1c")]),  # head 6
               (8, [lambda: ccout_load(1, (2, 6), "th1b")])]))

        # ---- tail: park partial FFN accumulations for all 8 th1 tiles in
        # the now-idle attention PSUM.  partA carries only (0,1,4,5); the
        # (2,6) fulls plus the th1c-halves (K=64, head 6 / peer head 14) of
        # ci3/ci7 then fill the last AllGather's latency, keeping the PE
        # warm; after th1d lands only the K=64 bottom halves (head 7 / peer
        # head 15) and the relu+store remain.
        CIS_A = (0, 1, 4, 5)
        hosts = []

        def host_partA(ot, pool, tag):
            if pool is flex_pool:
                h0 = pool.tile([P, 512], f32, tag=tag, name=f"tf{ot}a")
                h1 = pool.tile([P, 512], f32, tag=tag, name=f"tf{ot}b")
                pair = ((h0, 0), (h1, 0))
            else:
                ht = pool.tile([P, THALF], f32, tag=tag, name=f"tf{ot}")
                pair = ((ht, 0), (ht, 512))
            for tc, (ht, c0) in enumerate(pair):
                ffn_mms(ht[:, c0:c0 + 512], 1, ot, tc, CIS_A, True, False)
                hosts.append((ot, tc, ht, c0))

        def ffn_half(ht, lo, hi, ot, tc, cis, stop):
            for k, ci in enumerate(cis):
                nc.tensor.matmul(
                    ht, lhsT=wfT_sb[lo:hi, ci, P * ot:P * (ot + 1)],
                    rhs=ccout_sb[lo:hi, 1, ci, 512 * tc:512 * (tc + 1)],
                    start=False, stop=(stop and k == len(cis) - 1))

        # flex hosts first (PE work while the DVE recip chain of unit (7,1)
        # completes), then the norm tail (zbp must take its sc slot BEFORE
        # the sc hosts, else the slot rotation deadlocks), then sc/av hosts
        host_partA(0, flex_pool, "flex")
        flush_norm()
        host_partA(1, sc_pool, "sc")
        host_partA(2, sc_pool, "sc")
        host_partA(3, av_pool, "av")
        # head 7 of th1: the only tail-exposed collective (128KB)
        allgather(1, 448, 512, "th1d")
        out_c = ag_state["th1c"].rearrange("(k p) t -> k p t", p=64)
        nc.sync.dma_start(out=ccout_sb[0:64, 1, 3, :], in_=out_c[0])
        nc.sync.dma_start(out=ccout_sb[0:64, 1, 7, :], in_=out_c[1])
        # AllGather-latency fill: (2,6) fulls, then ci3/ci7 top halves
        for ot, tc, ht, c0 in hosts:
            ffn_mms(ht[:, c0:c0 + 512], 1, ot, tc, (2, 6), False, False)
        for ot, tc, ht, c0 in hosts:
            ffn_half(ht[:, c0:c0 + 512], 0, 64, ot, tc, (3, 7), False)
        out_d = ag_state["th1d"].rearrange("(k p) t -> k p t", p=64)
        nc.scalar.dma_start(out=ccout_sb[64:P, 1, 3, :], in_=out_d[0])
        nc.scalar.dma_start(out=ccout_sb[64:P, 1, 7, :], in_=out_d[1])
        for ot, tc, ht, c0 in hosts:
            ffn_half(ht[:, c0:c0 + 512], 64, P, ot, tc, (3, 7), True)
            ffn_out(ht[:, c0:c0 + 512], 1, ot, tc)

    nc.compile()
    return nc


def make_in_maps(x, Wq, Wk, Wv, Wf, bf):
    x = np.asarray(x, np.float32)
    mku_m = np.ascontiguousarray(
        np.triu(np.ones((P, P), np.float32))).astype(bf16)
    tri = (np.triu(np.ones((P, P), np.float32)) - 1.0) * MASK_BIG
    mkb_m = np.ascontiguousarray(np.concatenate(
        [np.full((P, P), -MASK_BIG, np.float32), tri], axis=1)).astype(bf16)
    bf_f = np.asarray(bf, np.float32)
    wfT_f = np.asarray(Wf, np.float32).T
    xw_dt = f8e4 if FP8_QKV else bf16
    ws = W_SCALE if FP8_QKV else 1.0
    in_maps = []
    for core in range(8):
        b, p = divmod(core, 2)
        sl = slice(HPC * p, HPC * (p + 1))
        in_maps.append({
            "xT": np.ascontiguousarray(x[b].T).astype(xw_dt),
            "wq": np.ascontiguousarray(
                np.asarray(Wq, np.float32)[:, sl].reshape(C, HPC * D)
                * ws).astype(xw_dt),
            "wk": np.ascontiguousarray(
                np.asarray(Wk, np.float32)[:, sl].reshape(C, HPC * D)
                * ws).astype(xw_dt),
            "wv": np.ascontiguousarray(
                np.asarray(Wv, np.float32)[:, sl].reshape(C, HPC * D)
                * ws).astype(xw_dt),
            "wfT": np.ascontiguousarray(
                wfT_f[:, COH * p:COH * (p + 1)]).astype(bf16),
            "mku": mku_m,
            "mkb": mkb_m,
            "biasco": np.ascontiguousarray(
                bf_f[COH * p:COH * (p + 1)].reshape(NOT, P).T),
        })
    return in_maps


def run(x, Wq, Wk, Wv, Wf, bf, trace=False, **spmd_kwargs):
    from concourse.bass_utils import run_bass_kernel_spmd

    if "nc" not in _CACHE:
        _CACHE["nc"] = build_nc()
    nc = _CACHE["nc"]
    in_maps = make_in_maps(x, Wq, Wk, Wv, Wf, bf)
    res = run_bass_kernel_spmd(
        nc, in_maps, core_ids=list(range(8)), trace=trace, **spmd_kwargs)
    out = np.zeros((B, T, C), np.float32)
    for core in range(8):
        b, p = divmod(core, 2)
        out[b, :, COH * p:COH * (p + 1)] = \
            res.results[core]["y"].T.astype(np.float32)
    return out, res


def kernel(x, Wq, Wk, Wv, Wf, bf):
    try:
        out, _ = run(x, Wq, Wk, Wv, Wf, bf, trace=False)
    except Exception:
        # transient device flake (hang/unrecoverable): one retry
        import time
        time.sleep(2.0)
        out, _ = run(x, Wq, Wk, Wv, Wf, bf, trace=False)
    return out



# revision 32
# speedup vs baseline: 1.0845x; 1.0262x over previous
"""Trainium2 Bass kernel for a dense transformer block (attention + ReLU FFN).

Reference computation (B=4, T=2048, C=1024, H=16, D=64):
    q,k,v = per-head projections of x;  causal softmax(q k^T / sqrt(C)) v;
    concat heads;  y = relu(out @ Wf.T + bf)

Sharding over 8 NeuronCores: core (2b+p) handles batch b with heads
[8p, 8p+8).  Attention runs causally over the full T on each core.  Pair
AllGathers (cores 2b/2b+1) share the attention outputs, and each core
runs the FFN for all 2048 tokens over its own half of the output
channels (the channel split is carried entirely by per-core input data -
every core executes an identical NEFF).

Layouts: scores are computed transposed ([s, t], keys on partitions) so
the exp() output feeds the AV matmul directly; V carries a PREPENDED
ones-column so row 0 of the AV accumulator is the softmax denominator
(landing on partition 0 where the custom-DVE reciprocal needs it);
causal masking zeroes the diagonal block's lower triangle on the DVE
after exp (a 0/1 triu multiply).  The 1/Z broadcast across partitions
is a rank-1 matmul into PSUM (a DMA broadcast costs ~11us of latency).
The FFN computes y transposed ([co, t]) so the bias+relu fuse into a
single scalar-engine activation (bias is per-partition); the host
transposes the per-core [COH, T] result back.

Engine budget: PE ~232us of streamed columns is the floor; ACT carries
exp+avc-evac+relu (~180us); DVE only does PSUM-evac casts + reciprocal;
GpSimd does the normalize multiply + broadcasts + collective triggers.
Collectives are 5 pair-AllGathers (two 512-row for th0, one 512 + two
256 for th1) emitted as early as their heads complete so only the last
(256 rows) is tail-exposed.  Compute dtype bf16 with fp32 PSUM
accumulation.
"""

import os
import sys

import numpy as np
import ml_dtypes

# Defensive: reset wedged NeuronCores on first init (must be set before the
# runtime initializes; a prior crashed process can leave cores unrecoverable)
os.environ.setdefault("NEURON_RT_RESET_CORES", "1")

for _p in ("/opt/trn_rl_repo", "/root/.axon_site/_ro/trn_rl_repo"):
    if os.path.isdir(_p) and _p not in sys.path:
        sys.path.append(_p)

B, T, C, H, D = 4, 2048, 1024, 16, 64
P = 128           # partitions
NCT = C // P      # 8 c-tiles
NTT = T // P      # 16 s/t-tiles
HPC = H // 2      # 8 heads per core
THALF = T // 2    # tokens per AllGather half
COH = C // 2      # output channels per core in the FFN
NOT = COH // P    # 4 co-tiles
SCALE = float(C) ** -0.5

# fp8 DoubleRow for the QKV projections: x and Wq/Wk/Wv ship as fp8e4m3
# (host pre-scales the weights by W_SCALE to clear the subnormal zone; the
# q/k scale cancels in the exp argument, the v scale cancels against the
# denominator because the ones-column is also W_SCALE).  Halves both the
# projection matmul streaming time and the input HBM traffic.
FP8_QKV = False
W_SCALE = 16.0
# q and k each carry a factor of W_SCALE under FP8_QKV; fold it out of the
# score scaling inside the exp's free affine
EXP_SCALE = SCALE / (W_SCALE * W_SCALE) if FP8_QKV else SCALE

# fp8 DoubleRow for the AV matmul: the exp writes fp8 weights for key-block
# PAIRS ([P, 2, THALF] tiles) and one DoubleRow matmul contracts both blocks
# (virtual K=256), halving the AV streaming time.  Causal masking moves
# PRE-exp: an additive -BIG mask on the fp32 scores (the DVE then never
# touches fp8), so exp emits exact zeros below the diagonal.
FP8_AV = False
MASK_BIG = 1.0e6

bf16 = ml_dtypes.bfloat16
f8e4 = ml_dtypes.float8_e4m3

_CACHE = {}


def build_nc():
    import concourse.bass as bass
    import concourse.tile as tile
    from concourse import bacc, mybir

    f32 = mybir.dt.float32
    b16 = mybir.dt.bfloat16
    f8 = mybir.dt.float8e4
    xdt = f8 if FP8_QKV else b16
    DR = mybir.MatmulPerfMode.DoubleRow
    EXP = mybir.ActivationFunctionType.Exp
    RELU = mybir.ActivationFunctionType.Relu

    nc = bacc.Bacc("TRN2", target_bir_lowering=False, debug=False, num_devices=8)

    xT = nc.dram_tensor("xT", [C, T], xdt, kind="ExternalInput").ap()
    wq = nc.dram_tensor("wq", [C, HPC * D], xdt, kind="ExternalInput").ap()
    wk = nc.dram_tensor("wk", [C, HPC * D], xdt, kind="ExternalInput").ap()
    wv = nc.dram_tensor("wv", [C, HPC * D], xdt, kind="ExternalInput").ap()
    wfT = nc.dram_tensor("wfT", [C, COH], b16, kind="ExternalInput").ap()
    mku = nc.dram_tensor("mku", [P, P], b16, kind="ExternalInput").ap()
    mkb = nc.dram_tensor("mkb", [P, 2 * P], b16, kind="ExternalInput").ap()
    biasco = nc.dram_tensor("biasco", [P, NOT], f32, kind="ExternalInput").ap()
    y = nc.dram_tensor("y", [COH, T], b16, kind="ExternalOutput").ap()

    RG = [[0, 1], [2, 3], [4, 5], [6, 7]]

    with tile.TileContext(nc) as tc, \
            tc.tile_pool(name="consts", bufs=1) as consts, \
            tc.tile_pool(name="dram", bufs=1, space="DRAM") as dram, \
            tc.tile_pool(name="sc_ps", bufs=2, space="PSUM") as sc_pool, \
            tc.tile_pool(name="av_ps", bufs=1, space="PSUM") as av_pool, \
            tc.tile_pool(name="flex_ps", bufs=2, space="PSUM") as flex_pool, \
            tc.tile_pool(name="wt", bufs=3) as wt_pool, \
            tc.tile_pool(name="norm", bufs=3) as norm_pool, \
            tc.tile_pool(name="yout", bufs=3) as y_pool:

        xT_sb = consts.tile([P, NCT, T], xdt)
        wq_sb = consts.tile([P, NCT, HPC * D], xdt)
        wk_sb = consts.tile([P, NCT, HPC * D], xdt)
        wv_sb = consts.tile([P, NCT, HPC * D], xdt)
        wfT_sb = consts.tile([P, NCT, COH], b16)
        mku_sb = consts.tile([P, P], b16)
        mkb_sb = consts.tile([P, 2 * P], b16)
        biasco_sb = consts.tile([P, NOT], f32)
        qT_sb = consts.tile([P, HPC // 2, T], b16)
        kT_sb = consts.tile([P, HPC // 2, T], b16)
        # fp8 AV: 66-wide v rows (ones-col + 64 data + a zero pad column so
        # the DoubleRow k-pair stride is 16B-aligned); bf16 AV: 65-wide
        if FP8_AV:
            v_sb = consts.tile([P, NTT, HPC, 2 + D], f8, name="v_sb")
        else:
            v_sb = consts.tile([P, NTT, HPC, 1 + D], b16, name="v_sb")
        ccout_sb = consts.tile([P, 2, NCT, THALF], b16)
        warm_sb = consts.tile([P, 8], f32)
        ones_sb = consts.tile([1, 1 + D], b16)

        cc_in = [dram.tile([HPC * D, THALF], b16, name=f"cc_in{i}", tag=f"cc_in{i}")
                 for i in (0, 1)]

        nc.vector.memset(warm_sb, 0.0)
        nc.vector.memset(ones_sb, 1.0)
        # the ones-column must carry the same scale as the fp8-projected v
        # rows so the softmax denominator (AV row 0) divides it out exactly
        nc.vector.memset(v_sb[:, :, :, 0:1], W_SCALE if FP8_QKV else 1.0)
        if FP8_AV:
            nc.vector.memset(v_sb[:, :, :, 1 + D:2 + D], 0.0)
            # zero the two score-PSUM ring slots once: the paired exp reads
            # masked-stale columns there, which must be finite (not NaN/Inf)
            for i in (0, 1):
                z = sc_pool.tile([P, THALF], f32, tag="sc", name=f"scz{i}")
                nc.vector.memset(z, 0.0)

        # ---- constant loads, spread over four DMA queues so the first QK
        # projection chunks are fed within ~5us -----------------------------
        xT_r = xT.rearrange("(ct p) t -> ct p t", p=P)
        wq_r = wq.rearrange("(ct p) m -> ct p m", p=P)
        wk_r = wk.rearrange("(ct p) m -> ct p m", p=P)
        wv_r = wv.rearrange("(ct p) m -> ct p m", p=P)
        wfT_r = wfT.rearrange("(ct p) co -> ct p co", p=P)
        xT_r2 = xT.rearrange("(cp p) t -> cp p t", p=2 * P)
        wq_r2 = wq.rearrange("(cp p) m -> cp p m", p=2 * P)
        wk_r2 = wk.rearrange("(cp p) m -> cp p m", p=2 * P)
        wv_r2 = wv.rearrange("(cp p) m -> cp p m", p=2 * P)
        for ct in range(NCT):
            nc.scalar.dma_start(out=wq_sb[:, ct, :], in_=wq_r[ct])
            nc.gpsimd.dma_start(out=wk_sb[:, ct, :], in_=wk_r[ct])
            (nc.sync if ct % 2 == 0 else nc.scalar).dma_start(
                out=xT_sb[:, ct, 0:THALF], in_=xT_r[ct][:, 0:THALF])
        nc.scalar.dma_start(out=mku_sb, in_=mku)
        for ct in range(NCT):
            nc.gpsimd.dma_start(out=wv_sb[:, ct, :], in_=wv_r[ct])
        if FP8_AV:
            nc.gpsimd.dma_start(out=mkb_sb, in_=mkb)
        for ct in range(NCT):
            (nc.sync if ct % 2 == 0 else nc.gpsimd).dma_start(
                out=xT_sb[:, ct, THALF:T], in_=xT_r[ct][:, THALF:T])
        for ct in range(NCT):
            nc.sync.dma_start(out=wfT_sb[:, ct, :], in_=wfT_r[ct])
        nc.sync.dma_start(out=biasco_sb, in_=biasco)
        # ACT exp-table preload (~2.7us, after the weight DMAs so it never
        # delays them; still long before the first real exp)
        nc.scalar.activation(out=warm_sb, in_=warm_sb, func=EXP)

        # ---- emission helpers ----------------------------------------------
        def qk_chunk(hp, which, g):
          with nc.named_scope("qkproj"):
            dst, w_t = ((qT_sb, wq_sb), (kT_sb, wk_sb))[which]
            ps = flex_pool.tile([P, 512], f32, tag="flex", name=f"qkps{hp}_{which}_{g}")
            if FP8_QKV:
                for c in range(NCT // 2):
                    nc.tensor.matmul(
                        ps, lhsT=w_t[:, 2 * c:2 * c + 2, hp * P:(hp + 1) * P],
                        rhs=xT_sb[:, 2 * c:2 * c + 2, 512 * g:512 * (g + 1)],
                        start=(c == 0), stop=(c == NCT // 2 - 1), perf_mode=DR)
            else:
                for ct in range(NCT):
                    nc.tensor.matmul(
                        ps, lhsT=w_t[:, ct, hp * P:(hp + 1) * P],
                        rhs=xT_sb[:, ct, 512 * g:512 * (g + 1)],
                        start=(ct == 0), stop=(ct == NCT - 1))
            nc.vector.tensor_copy(out=dst[:, hp, 512 * g:512 * (g + 1)], in_=ps)

        def v_proj(st):
          with nc.named_scope("vproj"):
            ps = flex_pool.tile([P, 512], f32, tag="flex", name=f"vps{st}")
            if FP8_QKV:
                for c in range(NCT // 2):
                    nc.tensor.matmul(
                        ps, lhsT=xT_sb[:, 2 * c:2 * c + 2, P * st:P * (st + 1)],
                        rhs=wv_sb[:, 2 * c:2 * c + 2, :],
                        start=(c == 0), stop=(c == NCT // 2 - 1), perf_mode=DR)
            else:
                for ct in range(NCT):
                    nc.tensor.matmul(
                        ps, lhsT=xT_sb[:, ct, P * st:P * (st + 1)],
                        rhs=wv_sb[:, ct, :],
                        start=(ct == 0), stop=(ct == NCT - 1))
            nc.vector.tensor_copy(out=v_sb[:, st, :, 1:1 + D],
                                  in_=ps.rearrange("p (h d) -> p h d", d=D))

        pending_norm = []

        def flush_norm():
            while pending_norm:
                pending_norm.pop(0)()

        def attn_unit(h, th, mid=None):
          with nc.named_scope(f"attn{th}_{h}"):
            hp, qh = divmod(h, 2)
            base = 64 * qh
            t0 = THALF * th
            av = av_pool.tile([P, THALF], f32, tag="av", name=f"av{h}_{th}")
            jmax = 8 * th + 8
            last_j = {0: 8 * th + 3, 1: jmax - 1}
            last_m = {0: 4 * th + 1, 1: 4 * th + 3}
            pend = None  # weights awaiting their AV emission

            def emit_av(ent):
                if FP8_AV:
                    m, offm, wtp = ent
                    pieces = [(offm, 512), (512, 1024)] if offm < 512 \
                        else [(offm, 1024)]
                    for (o, e) in pieces:
                        region = 0 if o < 512 else 1
                        nc.tensor.matmul(
                            av[0:D + 2, o:e],
                            lhsT=v_sb[:, 2 * m:2 * m + 2, h, :],
                            rhs=wtp[:, :, o:e],
                            start=(m == 0), stop=(m == last_m[region]),
                            perf_mode=DR)
                else:
                    j, pieces, wt = ent
                    for (o, e) in pieces:
                        region = 0 if o < 512 else 1
                        nc.tensor.matmul(
                            av[0:D + 1, o:e], lhsT=v_sb[:, j, h, :],
                            rhs=wt[:, o:e],
                            start=(j == 0), stop=(j == last_j[region]))

            wtp = None
            for j in range(jmax):
                m, jj = divmod(j, 2)
                off = max(0, P * j - t0)
                offm = max(0, P * (j - jj) - t0)  # pair-base column
                diag = P * j >= t0
                pieces = [(off, 512), (512, 1024)] if off < 512 \
                    else [(off, 1024)]
                sc = sc_pool.tile([P, THALF], f32, tag="sc", name=f"sc{h}_{th}_{j}")
                for pi, (o, e) in enumerate(pieces):
                    nc.tensor.matmul(
                        sc[:, o:e],
                        lhsT=kT_sb[base:base + 64, hp, P * j:P * (j + 1)],
                        rhs=qT_sb[base:base + 64, hp, t0 + o:t0 + e],
                        start=True, stop=True)
                if FP8_AV:
                    # additive -BIG causal mask on the fp32 scores (pre-exp);
                    # the odd half also wipes [off-P, off) so the paired AV
                    # sees exact zeros for its not-yet-causal key block
                    if jj == 0:
                        wtp = wt_pool.tile([P, 2, THALF], f8, tag="wt",
                                           name=f"wt{h}_{th}_{m}")
                    if diag:
                        if jj == 0:
                            nc.vector.tensor_add(
                                out=sc[:, off:off + P],
                                in0=sc[:, off:off + P], in1=mkb_sb[:, P:2 * P])
                        else:
                            nc.vector.tensor_add(
                                out=sc[:, off - P:off + P],
                                in0=sc[:, off - P:off + P], in1=mkb_sb)
                    nc.scalar.activation(out=wtp[:, jj, offm:THALF],
                                         in_=sc[:, offm:THALF],
                                         func=EXP, scale=EXP_SCALE)
                    if pend is not None and jj == 0:
                        emit_av(pend)
                else:
                    wt = wt_pool.tile([P, THALF], b16, tag="wt",
                                      name=f"wt{h}_{th}_{j}")
                    nc.scalar.activation(out=wt[:, off:THALF],
                                         in_=sc[:, off:THALF],
                                         func=EXP, scale=EXP_SCALE)
                    if diag:  # causal mask: zero the diagonal block's lower
                        # triangle on the DVE (cheaper than a PE mask-matmul;
                        # NOT on gpsimd - that queue carries the AllGather
                        # triggers, which must never sit behind exp-waits)
                        nc.vector.tensor_mul(out=wt[:, off:off + P],
                                             in0=wt[:, off:off + P], in1=mku_sb)
                    if pend is not None:
                        emit_av(pend)
                if j == 2:
                    flush_norm()  # previous unit's zbp/stage, ~4us after its
                    # last AV so the PE never waits on the DVE recip chain
                if mid is not None and j in mid:
                    for f in mid[j]:
                        f()
                if FP8_AV:
                    if jj == 1:
                        pend = (m, offm, wtp)
                else:
                    pend = (j, pieces, wt)
            emit_av(pend)
            # mid-kernel units: evacuate the accumulator FIRST (the copy is
            # what frees the av PSUM slot the next unit's AV matmuls need;
            # av_pool has bufs=1).  Last unit: reciprocal first, straight
            # off the PSUM denominator row, so the tail zbp/stage/AllGather
            # chain unblocks ~2us earlier (no next unit to starve).
            avc = norm_pool.tile([D + 1, THALF], f32, tag="avc", name=f"avc{h}_{th}")
            zr = norm_pool.tile([1, THALF], f32, tag="zr", name=f"zr{h}_{th}")
            zrb = norm_pool.tile([1, THALF], b16, tag="zrb", name=f"zrb{h}_{th}")
            if (h, th) == (7, 1):
                nc.vector.reciprocal_approx_fast(out=zr, in_=av[0:1, 0:THALF])
                nc.vector.tensor_copy(out=zrb, in_=zr)
                nc.vector.tensor_copy(out=avc, in_=av[0:D + 1, 0:THALF])
            else:
                nc.vector.tensor_copy(out=avc, in_=av[0:D + 1, 0:THALF])
                nc.vector.reciprocal_approx_fast(out=zr, in_=avc[0:1, :])
                nc.vector.tensor_copy(out=zrb, in_=zr)

            def norm_tail(h=h, th=th, avc=avc, zrb=zrb):
                # broadcast 1/Z across partitions.  Mid-kernel units use the
                # otherwise-idle GpSimd engine (saves ~0.4us of PE per unit
                # and an sc-slot rotation); the last unit keeps the PE
                # rank-1-matmul broadcast, whose latency is lower, because
                # its chain feeds the tail-exposed AllGather.
                if (h, th) != (7, 1):
                    zbb = norm_pool.tile([1 + D, THALF], b16, tag="zbb",
                                         name=f"zbb{h}_{th}")
                    nc.gpsimd.partition_broadcast(zbb, zrb)
                    zb_in = zbb
                else:
                    zbp = sc_pool.tile([P, THALF], f32, tag="sc",
                                       name=f"zbp{h}_{th}")
                    for o in (0, 512):  # one MM per PSUM bank
                        nc.tensor.matmul(zbp[0:1 + D, o:o + 512], lhsT=ones_sb,
                                         rhs=zrb[:, o:o + 512],
                                         start=True, stop=True)
                    zb_in = zbp[0:1 + D, :]
                # row 0 of stage is Z/Z (garbage); the cc_in DMA skips it
                # (DMAs have no partition-base alignment constraint)
                stage = norm_pool.tile([1 + D, THALF], b16, tag="stage",
                                       name=f"st{h}_{th}")
                nc.vector.tensor_mul(out=stage, in0=avc, in1=zb_in)
                # the last unit's stage-out rides the gpsimd queue so it is
                # not stuck behind tail ccout loads on sync, and the AG
                # trigger directly follows it in its own queue
                q = nc.gpsimd if (h, th) == (7, 1) else nc.sync
                q.dma_start(out=cc_in[th][64 * h:64 * (h + 1), :],
                            in_=stage[1:1 + D, :])

            pending_norm.append(norm_tail)

        ag_state = {}

        def allgather(th, r0, r1, nm):
          # trigger only; the SBUF loads are emitted later (ccout_load) so
          # their CC-completion waits never sit ahead of later stage-out
          # DMAs in the sync queue
          with nc.named_scope(f"ag_{nm}"):
            out_t = dram.tile([2 * (r1 - r0), THALF], b16,
                              name=f"ag_{nm}", tag=f"ag_{nm}")
            nc.gpsimd.collective_compute(
                "AllGather", mybir.AluOpType.bypass, replica_groups=RG,
                ins=[cc_in[th][r0:r1, :].opt()], outs=[out_t.opt()])
            ag_state[nm] = out_t

        def ccout_load(th, cis, nm):
            out_r = ag_state[nm].rearrange("(k p) t -> k p t", p=P)
            for k, ci in enumerate(cis):
                nc.sync.dma_start(out=ccout_sb[:, th, ci, :], in_=out_r[k])

        y_r = y.rearrange("(ot p) t -> ot p t", p=P)

        FFN_CI_ORDER = (0, 1, 4, 5, 2, 6, 3, 7)  # AllGather arrival order

        def ffn_mms(ps, th, ot, tc, cis, start, stop):
            for k, ci in enumerate(cis):
                nc.tensor.matmul(
                    ps, lhsT=wfT_sb[:, ci, P * ot:P * (ot + 1)],
                    rhs=ccout_sb[:, th, ci, 512 * tc:512 * (tc + 1)],
                    start=(start and k == 0), stop=(stop and k == len(cis) - 1))

        def ffn_out(ps, th, ot, tc):
            ysb = y_pool.tile([P, 512], b16, tag="y", name=f"y{th}_{ot}_{tc}")
            nc.scalar.activation(out=ysb, in_=ps, func=RELU,
                                 bias=biasco_sb[:, ot:ot + 1])
            t0 = THALF * th
            nc.sync.dma_start(out=y_r[ot][:, t0 + 512 * tc:t0 + 512 * (tc + 1)],
                              in_=ysb)

        def ffn_tile(th, ot, tc):
          with nc.named_scope("ffn"):
            ps = flex_pool.tile([P, 512], f32, tag="flex", name=f"fps{th}_{ot}_{tc}")
            ffn_mms(ps, th, ot, tc, FFN_CI_ORDER, True, True)
            ffn_out(ps, th, ot, tc)

        # ---- emission order -------------------------------------------------
        def qk4(hp, gs):
            return [lambda w=w, g=g, hp=hp: qk_chunk(hp, w, g)
                    for g in gs for w in (0, 1)]

        def vshots(sts):
            return [lambda st=st: v_proj(st) for st in sts]

        def fshots(specs):
            return [lambda s=s: ffn_tile(*s) for s in specs]

        def mids(fs, js):
            return {j: [f] for j, f in zip(js, fs)}

        # startup: q/k for head-pair 0 over the th0 token columns + first v
        for f in qk4(0, (0, 1)) + vshots(range(4)):
            f()

        # th0 attention; projection chunks fill unit boundaries
        attn_unit(0, 0, mid=mids(vshots(range(4, 8)), (0, 1, 2, 3)))
        for f in qk4(1, (0, 1)):
            f()
        attn_unit(1, 0)
        for f in qk4(2, (0, 1)):
            f()
        attn_unit(2, 0)
        for f in qk4(3, (0, 1)):
            f()
        attn_unit(3, 0)
        for f in qk4(0, (2,)):
            f()
        attn_unit(4, 0, mid={3: [lambda: allgather(0, 0, 256, "th0a")]})
        for f in qk4(0, (3,)):
            f()
        attn_unit(5, 0)
        ccout_load(0, (0, 1, 4, 5), "th0a")
        v_proj(8)
        attn_unit(6, 0)
        v_proj(9)
        v_proj(10)
        attn_unit(7, 0)
        v_proj(11)

        # th1 attention; q/k th1 columns + th0 FFN tiles fill the units
        attn_unit(0, 1, mid=dict(
            list(mids(vshots(range(12, 16)) + qk4(1, (2, 3)),
                      (0, 1, 4, 5, 7, 9, 11, 13)).items())
            + [(3, [lambda: allgather(0, 256, 512, "th0b")])]))
        ccout_load(0, (2, 3, 6, 7), "th0b")
        attn_unit(1, 1, mid=mids(qk4(2, (2, 3)), (1, 4, 6, 8)))
        attn_unit(2, 1, mid=mids(fshots([(0, 0, 0)]), (6,)))
        attn_unit(3, 1, mid=mids(fshots([(0, 0, 1)]), (6,)))
        attn_unit(4, 1, mid=dict(
            list(mids(fshots([(0, 1, 0)]) + qk4(3, (2,)), (6, 9, 12)).items())
            + [(3, [lambda: allgather(1, 0, 256, "th1a")])]))
        attn_unit(5, 1, mid=mids(fshots([(0, 1, 1)]) + qk4(3, (3,)), (4, 8, 11)))
        attn_unit(6, 1, mid=dict(
            list(mids(fshots([(0, 2, 0), (0, 2, 1)]), (6, 10)).items())
            + [(3, [lambda: allgather(1, 256, 384, "th1b")])]))
        ccout_load(1, (0, 1, 4, 5), "th1a")
        attn_unit(7, 1, mid=dict(
            list(mids(fshots([(0, 3, 0), (0, 3, 1)]), (6, 11)).items())
            + [(3, [lambda: allgather(1, 384, 448, "th1c")]),  # head 6
               (8, [lambda: ccout_load(1, (2, 6), "th1b")])]))

        # ---- tail: park partial FFN accumulations for all 8 th1 tiles in
        # the now-idle attention PSUM.  partA carries only (0,1,4,5); the
        # (2,6) fulls plus the th1c-halves (K=64, head 6 / peer head 14) of
        # ci3/ci7 then fill the last AllGather's latency, keeping the PE
        # warm; after th1d lands only the K=64 bottom halves (head 7 / peer
        # head 15) and the relu+store remain.
        CIS_A = (0, 1, 4, 5)
        hosts = []

        def host_partA(ot, pool, tag):
            if pool is flex_pool:
                h0 = pool.tile([P, 512], f32, tag=tag, name=f"tf{ot}a")
                h1 = pool.tile([P, 512], f32, tag=tag, name=f"tf{ot}b")
                pair = ((h0, 0), (h1, 0))
            else:
                ht = pool.tile([P, THALF], f32, tag=tag, name=f"tf{ot}")
                pair = ((ht, 0), (ht, 512))
            for tc, (ht, c0) in enumerate(pair):
                ffn_mms(ht[:, c0:c0 + 512], 1, ot, tc, CIS_A, True, False)
                hosts.append((ot, tc, ht, c0))

        def ffn_half(ht, lo, hi, ot, tc, cis, stop):
            for k, ci in enumerate(cis):
                nc.tensor.matmul(
                    ht, lhsT=wfT_sb[lo:hi, ci, P * ot:P * (ot + 1)],
                    rhs=ccout_sb[lo:hi, 1, ci, 512 * tc:512 * (tc + 1)],
                    start=False, stop=(stop and k == len(cis) - 1))

        # flex hosts first (PE work while the DVE recip chain of unit (7,1)
        # completes), then the norm tail (zbp must take its sc slot BEFORE
        # the sc hosts, else the slot rotation deadlocks), then sc/av hosts
        host_partA(0, flex_pool, "flex")
        flush_norm()
        host_partA(1, sc_pool, "sc")
        host_partA(2, sc_pool, "sc")
        host_partA(3, av_pool, "av")
        # head 7 of th1: the only tail-exposed collective (128KB)
        allgather(1, 448, 512, "th1d")
        out_c = ag_state["th1c"].rearrange("(k p) t -> k p t", p=64)
        nc.sync.dma_start(out=ccout_sb[0:64, 1, 3, :], in_=out_c[0])
        nc.sync.dma_start(out=ccout_sb[0:64, 1, 7, :], in_=out_c[1])
        # AllGather-latency fill: (2,6) fulls, then ci3/ci7 top halves
        for ot, tc, ht, c0 in hosts:
            ffn_mms(ht[:, c0:c0 + 512], 1, ot, tc, (2, 6), False, False)
        for ot, tc, ht, c0 in hosts:
            ffn_half(ht[:, c0:c0 + 512], 0, 64, ot, tc, (3, 7), False)
        out_d = ag_state["th1d"].rearrange("(k p) t -> k p t", p=64)
        nc.scalar.dma_start(out=ccout_sb[64:P, 1, 3, :], in_=out_d[0])
        nc.scalar.dma_start(out=ccout_sb[64:P, 1, 7, :], in_=out_d[1])
        for ot, tc, ht, c0 in hosts:
            ffn_half(ht[:, c0:c0 + 512], 64, P, ot, tc, (3, 7), True)
            ffn_out(ht[:, c0:c0 + 512], 1, ot, tc)

    nc.compile()
    return nc


def make_in_maps(x, Wq, Wk, Wv, Wf, bf):
    x = np.asarray(x, np.float32)
    mku_m = np.ascontiguousarray(
        np.triu(np.ones((P, P), np.float32))).astype(bf16)
    tri = (np.triu(np.ones((P, P), np.float32)) - 1.0) * MASK_BIG
    mkb_m = np.ascontiguousarray(np.concatenate(
        [np.full((P, P), -MASK_BIG, np.float32), tri], axis=1)).astype(bf16)
    bf_f = np.asarray(bf, np.float32)
    wfT_f = np.asarray(Wf, np.float32).T
    xw_dt = f8e4 if FP8_QKV else bf16
    ws = W_SCALE if FP8_QKV else 1.0
    in_maps = []
    for core in range(8):
        b, p = divmod(core, 2)
        sl = slice(HPC * p, HPC * (p + 1))
        in_maps.append({
            "xT": np.ascontiguousarray(x[b].T).astype(xw_dt),
            "wq": np.ascontiguousarray(
                np.asarray(Wq, np.float32)[:, sl].reshape(C, HPC * D)
                * ws).astype(xw_dt),
            "wk": np.ascontiguousarray(
                np.asarray(Wk, np.float32)[:, sl].reshape(C, HPC * D)
                * ws).astype(xw_dt),
            "wv": np.ascontiguousarray(
                np.asarray(Wv, np.float32)[:, sl].reshape(C, HPC * D)
                * ws).astype(xw_dt),
            "wfT": np.ascontiguousarray(
                wfT_f[:, COH * p:COH * (p + 1)]).astype(bf16),
            "mku": mku_m,
            "mkb": mkb_m,
            "biasco": np.ascontiguousarray(
                bf_f[COH * p:COH * (p + 1)].reshape(NOT, P).T),
        })
    return in_maps


def run(x, Wq, Wk, Wv, Wf, bf, trace=False, **spmd_kwargs):
    from concourse.bass_utils import run_bass_kernel_spmd

    if "nc" not in _CACHE:
        _CACHE["nc"] = build_nc()
    nc = _CACHE["nc"]
    in_maps = make_in_maps(x, Wq, Wk, Wv, Wf, bf)
    res = run_bass_kernel_spmd(
        nc, in_maps, core_ids=list(range(8)), trace=trace, **spmd_kwargs)
    out = np.zeros((B, T, C), np.float32)
    for core in range(8):
        b, p = divmod(core, 2)
        out[b, :, COH * p:COH * (p + 1)] = \
            res.results[core]["y"].T.astype(np.float32)
    return out, res


def kernel(x, Wq, Wk, Wv, Wf, bf):
    try:
        out, _ = run(x, Wq, Wk, Wv, Wf, bf, trace=False)
    except Exception:
        # transient device flake (hang/unrecoverable): one retry
        import time
        time.sleep(2.0)
        out, _ = run(x, Wq, Wk, Wv, Wf, bf, trace=False)
    return out



# revision 35
# speedup vs baseline: 1.1149x; 1.0280x over previous
"""Trainium2 Bass kernel for a dense transformer block (attention + ReLU FFN).

Reference computation (B=4, T=2048, C=1024, H=16, D=64):
    q,k,v = per-head projections of x;  causal softmax(q k^T / sqrt(C)) v;
    concat heads;  y = relu(out @ Wf.T + bf)

Sharding over 8 NeuronCores: core (2b+p) handles batch b with heads
[8p, 8p+8).  Attention runs causally over the full T on each core.  Pair
AllGathers (cores 2b/2b+1) share the attention outputs, and each core
runs the FFN for all 2048 tokens over its own half of the output
channels (the channel split is carried entirely by per-core input data -
every core executes an identical NEFF).

Layouts: scores are computed transposed ([s, t], keys on partitions) so
the exp() output feeds the AV matmul directly; V carries a PREPENDED
ones-column so row 0 of the AV accumulator is the softmax denominator
(landing on partition 0 where the custom-DVE reciprocal needs it);
causal masking zeroes the diagonal block's lower triangle on the DVE
after exp (a 0/1 triu multiply).  The 1/Z broadcast across partitions
is a rank-1 matmul into PSUM (a DMA broadcast costs ~11us of latency).
The FFN computes y transposed ([co, t]) so the bias+relu fuse into a
single scalar-engine activation (bias is per-partition); the host
transposes the per-core [COH, T] result back.

Engine budget: PE ~232us of streamed columns is the floor; ACT carries
exp+avc-evac+relu (~180us); DVE only does PSUM-evac casts + reciprocal;
GpSimd does the normalize multiply + broadcasts + collective triggers.
Collectives are 5 pair-AllGathers (two 512-row for th0, one 512 + two
256 for th1) emitted as early as their heads complete so only the last
(256 rows) is tail-exposed.  Compute dtype bf16 with fp32 PSUM
accumulation.
"""

import os
import sys

import numpy as np
import ml_dtypes

# Defensive: reset wedged NeuronCores on first init (must be set before the
# runtime initializes; a prior crashed process can leave cores unrecoverable)
os.environ.setdefault("NEURON_RT_RESET_CORES", "1")

for _p in ("/opt/trn_rl_repo", "/root/.axon_site/_ro/trn_rl_repo"):
    if os.path.isdir(_p) and _p not in sys.path:
        sys.path.append(_p)

B, T, C, H, D = 4, 2048, 1024, 16, 64
P = 128           # partitions
NCT = C // P      # 8 c-tiles
NTT = T // P      # 16 s/t-tiles
HPC = H // 2      # 8 heads per core
THALF = T // 2    # tokens per AllGather half
COH = C // 2      # output channels per core in the FFN
NOT = COH // P    # 4 co-tiles
SCALE = float(C) ** -0.5

# fp8 DoubleRow for the QKV projections: x and Wq/Wk/Wv ship as fp8e4m3
# (host pre-scales the weights by W_SCALE to clear the subnormal zone; the
# q/k scale cancels in the exp argument, the v scale cancels against the
# denominator because the ones-column is also W_SCALE).  Halves both the
# projection matmul streaming time and the input HBM traffic.
FP8_QKV = False
W_SCALE = 16.0
# q and k each carry a factor of W_SCALE under FP8_QKV; fold it out of the
# score scaling inside the exp's free affine
EXP_SCALE = SCALE / (W_SCALE * W_SCALE) if FP8_QKV else SCALE

# fp8 DoubleRow for the AV matmul: the exp writes fp8 weights for key-block
# PAIRS ([P, 2, THALF] tiles) and one DoubleRow matmul contracts both blocks
# (virtual K=256), halving the AV streaming time.  Causal masking moves
# PRE-exp: an additive -BIG mask on the fp32 scores (the DVE then never
# touches fp8), so exp emits exact zeros below the diagonal.
FP8_AV = False
MASK_BIG = 1.0e6

bf16 = ml_dtypes.bfloat16
f8e4 = ml_dtypes.float8_e4m3

_CACHE = {}


def build_nc():
    import concourse.bass as bass
    import concourse.tile as tile
    from concourse import bacc, mybir

    f32 = mybir.dt.float32
    b16 = mybir.dt.bfloat16
    f8 = mybir.dt.float8e4
    xdt = f8 if FP8_QKV else b16
    DR = mybir.MatmulPerfMode.DoubleRow
    EXP = mybir.ActivationFunctionType.Exp
    RELU = mybir.ActivationFunctionType.Relu

    nc = bacc.Bacc("TRN2", target_bir_lowering=False, debug=False, num_devices=8)

    xT = nc.dram_tensor("xT", [C, T], xdt, kind="ExternalInput").ap()
    wq = nc.dram_tensor("wq", [C, HPC * D], xdt, kind="ExternalInput").ap()
    wk = nc.dram_tensor("wk", [C, HPC * D], xdt, kind="ExternalInput").ap()
    wv = nc.dram_tensor("wv", [C, HPC * D], xdt, kind="ExternalInput").ap()
    wfT = nc.dram_tensor("wfT", [C, COH], b16, kind="ExternalInput").ap()
    mku = nc.dram_tensor("mku", [P, P], b16, kind="ExternalInput").ap()
    mkb = nc.dram_tensor("mkb", [P, 2 * P], b16, kind="ExternalInput").ap()
    biasco = nc.dram_tensor("biasco", [P, NOT], f32, kind="ExternalInput").ap()
    y = nc.dram_tensor("y", [COH, T], b16, kind="ExternalOutput").ap()

    RG = [[0, 1], [2, 3], [4, 5], [6, 7]]

    with tile.TileContext(nc) as tc, \
            tc.tile_pool(name="consts", bufs=1) as consts, \
            tc.tile_pool(name="dram", bufs=1, space="DRAM") as dram, \
            tc.tile_pool(name="sc_ps", bufs=2, space="PSUM") as sc_pool, \
            tc.tile_pool(name="av_ps", bufs=1, space="PSUM") as av_pool, \
            tc.tile_pool(name="flex_ps", bufs=2, space="PSUM") as flex_pool, \
            tc.tile_pool(name="wt", bufs=3) as wt_pool, \
            tc.tile_pool(name="norm", bufs=3) as norm_pool, \
            tc.tile_pool(name="yout", bufs=3) as y_pool:

        xT_sb = consts.tile([P, NCT, T], xdt)
        wq_sb = consts.tile([P, NCT, HPC * D], xdt)
        wk_sb = consts.tile([P, NCT, HPC * D], xdt)
        wv_sb = consts.tile([P, NCT, HPC * D], xdt)
        wfT_sb = consts.tile([P, NCT, COH], b16)
        mku_sb = consts.tile([P, P], b16)
        mkb_sb = consts.tile([P, 2 * P], b16)
        biasco_sb = consts.tile([P, NOT], f32)
        qT_sb = consts.tile([P, HPC // 2, T], b16)
        kT_sb = consts.tile([P, HPC // 2, T], b16)
        # fp8 AV: 66-wide v rows (ones-col + 64 data + a zero pad column so
        # the DoubleRow k-pair stride is 16B-aligned); bf16 AV: 65-wide
        if FP8_AV:
            v_sb = consts.tile([P, NTT, HPC, 2 + D], f8, name="v_sb")
        else:
            v_sb = consts.tile([P, NTT, HPC, 1 + D], b16, name="v_sb")
        ccout_sb = consts.tile([P, 2, NCT, THALF], b16)
        warm_sb = consts.tile([P, 8], f32)
        ones_sb = consts.tile([1, 1 + D], b16)

        cc_in = [dram.tile([HPC * D, THALF], b16, name=f"cc_in{i}", tag=f"cc_in{i}")
                 for i in (0, 1)]

        nc.vector.memset(warm_sb, 0.0)
        nc.vector.memset(ones_sb, 1.0)
        # the ones-column must carry the same scale as the fp8-projected v
        # rows so the softmax denominator (AV row 0) divides it out exactly
        nc.vector.memset(v_sb[:, :, :, 0:1], W_SCALE if FP8_QKV else 1.0)
        if FP8_AV:
            nc.vector.memset(v_sb[:, :, :, 1 + D:2 + D], 0.0)
            # zero the two score-PSUM ring slots once: the paired exp reads
            # masked-stale columns there, which must be finite (not NaN/Inf)
            for i in (0, 1):
                z = sc_pool.tile([P, THALF], f32, tag="sc", name=f"scz{i}")
                nc.vector.memset(z, 0.0)

        # ---- constant loads, spread over four DMA queues so the first QK
        # projection chunks are fed within ~5us -----------------------------
        xT_r = xT.rearrange("(ct p) t -> ct p t", p=P)
        wq_r = wq.rearrange("(ct p) m -> ct p m", p=P)
        wk_r = wk.rearrange("(ct p) m -> ct p m", p=P)
        wv_r = wv.rearrange("(ct p) m -> ct p m", p=P)
        wfT_r = wfT.rearrange("(ct p) co -> ct p co", p=P)
        xT_r2 = xT.rearrange("(cp p) t -> cp p t", p=2 * P)
        wq_r2 = wq.rearrange("(cp p) m -> cp p m", p=2 * P)
        wk_r2 = wk.rearrange("(cp p) m -> cp p m", p=2 * P)
        wv_r2 = wv.rearrange("(cp p) m -> cp p m", p=2 * P)
        for ct in range(NCT):
            nc.scalar.dma_start(out=wq_sb[:, ct, :], in_=wq_r[ct])
            nc.gpsimd.dma_start(out=wk_sb[:, ct, :], in_=wk_r[ct])
            (nc.sync if ct % 2 == 0 else nc.scalar).dma_start(
                out=xT_sb[:, ct, 0:THALF], in_=xT_r[ct][:, 0:THALF])
        nc.scalar.dma_start(out=mku_sb, in_=mku)
        for ct in range(NCT):
            nc.gpsimd.dma_start(out=wv_sb[:, ct, :], in_=wv_r[ct])
        if FP8_AV:
            nc.gpsimd.dma_start(out=mkb_sb, in_=mkb)
        for ct in range(NCT):
            (nc.sync if ct % 2 == 0 else nc.gpsimd).dma_start(
                out=xT_sb[:, ct, THALF:T], in_=xT_r[ct][:, THALF:T])
        for ct in range(NCT):
            nc.sync.dma_start(out=wfT_sb[:, ct, :], in_=wfT_r[ct])
        nc.sync.dma_start(out=biasco_sb, in_=biasco)
        # ACT exp-table preload (~2.7us, after the weight DMAs so it never
        # delays them; still long before the first real exp)
        nc.scalar.activation(out=warm_sb, in_=warm_sb, func=EXP)

        # ---- emission helpers ----------------------------------------------
        def qk_chunk(hp, which, g):
          with nc.named_scope("qkproj"):
            dst, w_t = ((qT_sb, wq_sb), (kT_sb, wk_sb))[which]
            ps = flex_pool.tile([P, 512], f32, tag="flex", name=f"qkps{hp}_{which}_{g}")
            if FP8_QKV:
                for c in range(NCT // 2):
                    nc.tensor.matmul(
                        ps, lhsT=w_t[:, 2 * c:2 * c + 2, hp * P:(hp + 1) * P],
                        rhs=xT_sb[:, 2 * c:2 * c + 2, 512 * g:512 * (g + 1)],
                        start=(c == 0), stop=(c == NCT // 2 - 1), perf_mode=DR)
            else:
                for ct in range(NCT):
                    nc.tensor.matmul(
                        ps, lhsT=w_t[:, ct, hp * P:(hp + 1) * P],
                        rhs=xT_sb[:, ct, 512 * g:512 * (g + 1)],
                        start=(ct == 0), stop=(ct == NCT - 1))
            nc.vector.tensor_copy(out=dst[:, hp, 512 * g:512 * (g + 1)], in_=ps)

        def v_proj(st):
          with nc.named_scope("vproj"):
            ps = flex_pool.tile([P, 512], f32, tag="flex", name=f"vps{st}")
            if FP8_QKV:
                for c in range(NCT // 2):
                    nc.tensor.matmul(
                        ps, lhsT=xT_sb[:, 2 * c:2 * c + 2, P * st:P * (st + 1)],
                        rhs=wv_sb[:, 2 * c:2 * c + 2, :],
                        start=(c == 0), stop=(c == NCT // 2 - 1), perf_mode=DR)
            else:
                for ct in range(NCT):
                    nc.tensor.matmul(
                        ps, lhsT=xT_sb[:, ct, P * st:P * (st + 1)],
                        rhs=wv_sb[:, ct, :],
                        start=(ct == 0), stop=(ct == NCT - 1))
            nc.vector.tensor_copy(out=v_sb[:, st, :, 1:1 + D],
                                  in_=ps.rearrange("p (h d) -> p h d", d=D))

        pending_norm = []

        def flush_norm():
            while pending_norm:
                pending_norm.pop(0)()

        def attn_unit(h, th, mid=None):
          with nc.named_scope(f"attn{th}_{h}"):
            hp, qh = divmod(h, 2)
            base = 64 * qh
            t0 = THALF * th
            av = av_pool.tile([P, THALF], f32, tag="av", name=f"av{h}_{th}")
            jmax = 8 * th + 8
            last_j = {0: 8 * th + 3, 1: jmax - 1}
            last_m = {0: 4 * th + 1, 1: 4 * th + 3}
            pend = None   # fp8 path: weights awaiting their AV emission
            pend2 = []    # bf16 path: up-to-two j's awaiting AV emission

            def emit_av(ent):
                if FP8_AV:
                    m, offm, wtp = ent
                    pieces = [(offm, 512), (512, 1024)] if offm < 512 \
                        else [(offm, 1024)]
                    for (o, e) in pieces:
                        region = 0 if o < 512 else 1
                        nc.tensor.matmul(
                            av[0:D + 2, o:e],
                            lhsT=v_sb[:, 2 * m:2 * m + 2, h, :],
                            rhs=wtp[:, :, o:e],
                            start=(m == 0), stop=(m == last_m[region]),
                            perf_mode=DR)
                else:
                    j, pieces, wt = ent
                    for (o, e) in pieces:
                        region = 0 if o < 512 else 1
                        nc.tensor.matmul(
                            av[0:D + 1, o:e], lhsT=v_sb[:, j, h, :],
                            rhs=wt[:, o:e],
                            start=(j == 0), stop=(j == last_j[region]))

            wtp = None
            for j in range(jmax):
                m, jj = divmod(j, 2)
                off = max(0, P * j - t0)
                offm = max(0, P * (j - jj) - t0)  # pair-base column
                diag = P * j >= t0
                pieces = [(off, 512), (512, 1024)] if off < 512 \
                    else [(off, 1024)]
                sc = sc_pool.tile([P, THALF], f32, tag="sc", name=f"sc{h}_{th}_{j}")
                for pi, (o, e) in enumerate(pieces):
                    nc.tensor.matmul(
                        sc[:, o:e],
                        lhsT=kT_sb[base:base + 64, hp, P * j:P * (j + 1)],
                        rhs=qT_sb[base:base + 64, hp, t0 + o:t0 + e],
                        start=True, stop=True)
                if FP8_AV:
                    # additive -BIG causal mask on the fp32 scores (pre-exp);
                    # the odd half also wipes [off-P, off) so the paired AV
                    # sees exact zeros for its not-yet-causal key block
                    if jj == 0:
                        wtp = wt_pool.tile([P, 2, THALF], f8, tag="wt",
                                           name=f"wt{h}_{th}_{m}")
                    if diag:
                        if jj == 0:
                            nc.vector.tensor_add(
                                out=sc[:, off:off + P],
                                in0=sc[:, off:off + P], in1=mkb_sb[:, P:2 * P])
                        else:
                            nc.vector.tensor_add(
                                out=sc[:, off - P:off + P],
                                in0=sc[:, off - P:off + P], in1=mkb_sb)
                    nc.scalar.activation(out=wtp[:, jj, offm:THALF],
                                         in_=sc[:, offm:THALF],
                                         func=EXP, scale=EXP_SCALE)
                    if pend is not None and jj == 0:
                        emit_av(pend)
                else:
                    wt = wt_pool.tile([P, THALF], b16, tag="wt",
                                      name=f"wt{h}_{th}_{j}")
                    nc.scalar.activation(out=wt[:, off:THALF],
                                         in_=sc[:, off:THALF],
                                         func=EXP, scale=EXP_SCALE)
                    if diag:  # causal mask: zero the diagonal block's lower
                        # triangle on the DVE (cheaper than a PE mask-matmul;
                        # NOT on gpsimd - that queue carries the AllGather
                        # triggers, which must never sit behind exp-waits)
                        nc.vector.tensor_mul(out=wt[:, off:off + P],
                                             in0=wt[:, off:off + P], in1=mku_sb)
                    # run the AV two j-steps behind its exp (wt_pool bufs=3
                    # keeps exactly 3 weight tiles alive) so the PE never
                    # waits on the ACT exp chain's per-instruction overhead
                    if len(pend2) == 2:
                        emit_av(pend2.pop(0))
                if j == 2:
                    flush_norm()  # previous unit's zbp/stage, ~4us after its
                    # last AV so the PE never waits on the DVE recip chain
                if mid is not None and j in mid:
                    for f in mid[j]:
                        f()
                if FP8_AV:
                    if jj == 1:
                        pend = (m, offm, wtp)
                else:
                    pend2.append((j, pieces, wt))
            if FP8_AV:
                emit_av(pend)
            else:
                for ent in pend2:
                    emit_av(ent)
            # mid-kernel units: evacuate the accumulator FIRST (the copy is
            # what frees the av PSUM slot the next unit's AV matmuls need;
            # av_pool has bufs=1).  Last unit: reciprocal first, straight
            # off the PSUM denominator row, so the tail zbp/stage/AllGather
            # chain unblocks ~2us earlier (no next unit to starve).
            avc = norm_pool.tile([D + 1, THALF], f32, tag="avc", name=f"avc{h}_{th}")
            zr = norm_pool.tile([1, THALF], f32, tag="zr", name=f"zr{h}_{th}")
            zrb = norm_pool.tile([1, THALF], b16, tag="zrb", name=f"zrb{h}_{th}")
            if (h, th) == (7, 1):
                nc.vector.reciprocal_approx_fast(out=zr, in_=av[0:1, 0:THALF])
                nc.vector.tensor_copy(out=zrb, in_=zr)
                nc.vector.tensor_copy(out=avc, in_=av[0:D + 1, 0:THALF])
            else:
                nc.vector.tensor_copy(out=avc, in_=av[0:D + 1, 0:THALF])
                nc.vector.reciprocal_approx_fast(out=zr, in_=avc[0:1, :])
                nc.vector.tensor_copy(out=zrb, in_=zr)

            def norm_tail(h=h, th=th, avc=avc, zrb=zrb):
                # broadcast 1/Z across partitions.  Mid-kernel units use the
                # otherwise-idle GpSimd engine (saves ~0.4us of PE per unit
                # and an sc-slot rotation); the last unit keeps the PE
                # rank-1-matmul broadcast, whose latency is lower, because
                # its chain feeds the tail-exposed AllGather.
                if (h, th) != (7, 1):
                    zbb = norm_pool.tile([1 + D, THALF], b16, tag="zbb",
                                         name=f"zbb{h}_{th}")
                    nc.gpsimd.partition_broadcast(zbb, zrb)
                    zb_in = zbb
                else:
                    zbp = sc_pool.tile([P, THALF], f32, tag="sc",
                                       name=f"zbp{h}_{th}")
                    for o in (0, 512):  # one MM per PSUM bank
                        nc.tensor.matmul(zbp[0:1 + D, o:o + 512], lhsT=ones_sb,
                                         rhs=zrb[:, o:o + 512],
                                         start=True, stop=True)
                    zb_in = zbp[0:1 + D, :]
                # row 0 of stage is Z/Z (garbage); the cc_in DMA skips it
                # (DMAs have no partition-base alignment constraint)
                stage = norm_pool.tile([1 + D, THALF], b16, tag="stage",
                                       name=f"st{h}_{th}")
                nc.vector.tensor_mul(out=stage, in0=avc, in1=zb_in)
                # the last unit's stage-out rides the gpsimd queue so it is
                # not stuck behind tail ccout loads on sync, and the AG
                # trigger directly follows it in its own queue
                q = nc.gpsimd if (h, th) == (7, 1) else nc.sync
                q.dma_start(out=cc_in[th][64 * h:64 * (h + 1), :],
                            in_=stage[1:1 + D, :])

            pending_norm.append(norm_tail)

        ag_state = {}

        def allgather(th, r0, r1, nm):
          # trigger only; the SBUF loads are emitted later (ccout_load) so
          # their CC-completion waits never sit ahead of later stage-out
          # DMAs in the sync queue
          with nc.named_scope(f"ag_{nm}"):
            out_t = dram.tile([2 * (r1 - r0), THALF], b16,
                              name=f"ag_{nm}", tag=f"ag_{nm}")
            nc.gpsimd.collective_compute(
                "AllGather", mybir.AluOpType.bypass, replica_groups=RG,
                ins=[cc_in[th][r0:r1, :].opt()], outs=[out_t.opt()])
            ag_state[nm] = out_t

        def ccout_load(th, cis, nm):
            out_r = ag_state[nm].rearrange("(k p) t -> k p t", p=P)
            for k, ci in enumerate(cis):
                nc.sync.dma_start(out=ccout_sb[:, th, ci, :], in_=out_r[k])

        y_r = y.rearrange("(ot p) t -> ot p t", p=P)

        FFN_CI_ORDER = (0, 1, 4, 5, 2, 6, 3, 7)  # AllGather arrival order

        def ffn_mms(ps, th, ot, tc, cis, start, stop):
            for k, ci in enumerate(cis):
                nc.tensor.matmul(
                    ps, lhsT=wfT_sb[:, ci, P * ot:P * (ot + 1)],
                    rhs=ccout_sb[:, th, ci, 512 * tc:512 * (tc + 1)],
                    start=(start and k == 0), stop=(stop and k == len(cis) - 1))

        def ffn_out(ps, th, ot, tc):
            ysb = y_pool.tile([P, 512], b16, tag="y", name=f"y{th}_{ot}_{tc}")
            nc.scalar.activation(out=ysb, in_=ps, func=RELU,
                                 bias=biasco_sb[:, ot:ot + 1])
            t0 = THALF * th
            nc.sync.dma_start(out=y_r[ot][:, t0 + 512 * tc:t0 + 512 * (tc + 1)],
                              in_=ysb)

        def ffn_tile(th, ot, tc):
          with nc.named_scope("ffn"):
            ps = flex_pool.tile([P, 512], f32, tag="flex", name=f"fps{th}_{ot}_{tc}")
            ffn_mms(ps, th, ot, tc, FFN_CI_ORDER, True, True)
            ffn_out(ps, th, ot, tc)

        # ---- emission order -------------------------------------------------
        def qk4(hp, gs):
            return [lambda w=w, g=g, hp=hp: qk_chunk(hp, w, g)
                    for g in gs for w in (0, 1)]

        def vshots(sts):
            return [lambda st=st: v_proj(st) for st in sts]

        def fshots(specs):
            return [lambda s=s: ffn_tile(*s) for s in specs]

        def mids(fs, js):
            return {j: [f] for j, f in zip(js, fs)}

        # startup: q/k for head-pair 0 over the th0 token columns + first v
        for f in qk4(0, (0, 1)) + vshots(range(4)):
            f()

        # th0 attention; projection chunks fill unit boundaries
        attn_unit(0, 0, mid=mids(vshots(range(4, 8)), (0, 1, 2, 3)))
        for f in qk4(1, (0, 1)):
            f()
        attn_unit(1, 0)
        for f in qk4(2, (0, 1)):
            f()
        attn_unit(2, 0)
        for f in qk4(3, (0, 1)):
            f()
        attn_unit(3, 0)
        for f in qk4(0, (2,)):
            f()
        attn_unit(4, 0, mid={3: [lambda: allgather(0, 0, 256, "th0a")]})
        for f in qk4(0, (3,)):
            f()
        attn_unit(5, 0)
        ccout_load(0, (0, 1, 4, 5), "th0a")
        v_proj(8)
        attn_unit(6, 0)
        v_proj(9)
        v_proj(10)
        attn_unit(7, 0)
        v_proj(11)

        # th1 attention; q/k th1 columns + th0 FFN tiles fill the units
        attn_unit(0, 1, mid=dict(
            list(mids(vshots(range(12, 16)) + qk4(1, (2, 3)),
                      (0, 1, 4, 5, 7, 9, 11, 13)).items())
            + [(3, [lambda: allgather(0, 256, 512, "th0b")])]))
        ccout_load(0, (2, 3, 6, 7), "th0b")
        attn_unit(1, 1, mid=mids(qk4(2, (2, 3)), (1, 4, 6, 8)))
        attn_unit(2, 1, mid=mids(fshots([(0, 0, 0)]), (6,)))
        attn_unit(3, 1, mid=mids(fshots([(0, 0, 1)]), (6,)))
        attn_unit(4, 1, mid=dict(
            list(mids(fshots([(0, 1, 0)]) + qk4(3, (2,)), (6, 9, 12)).items())
            + [(3, [lambda: allgather(1, 0, 256, "th1a")])]))
        attn_unit(5, 1, mid=mids(fshots([(0, 1, 1)]) + qk4(3, (3,)), (4, 8, 11)))
        attn_unit(6, 1, mid=dict(
            list(mids(fshots([(0, 2, 0), (0, 2, 1)]), (6, 10)).items())
            + [(3, [lambda: allgather(1, 256, 384, "th1b")])]))
        ccout_load(1, (0, 1, 4, 5), "th1a")
        attn_unit(7, 1, mid=dict(
            list(mids(fshots([(0, 3, 0), (0, 3, 1)]), (6, 11)).items())
            + [(3, [lambda: allgather(1, 384, 448, "th1c")]),  # head 6
               (8, [lambda: ccout_load(1, (2, 6), "th1b")])]))

        # ---- tail: park partial FFN accumulations for all 8 th1 tiles in
        # the now-idle attention PSUM.  partA carries only (0,1,4,5); the
        # (2,6) fulls plus the th1c-halves (K=64, head 6 / peer head 14) of
        # ci3/ci7 then fill the last AllGather's latency, keeping the PE
        # warm; after th1d lands only the K=64 bottom halves (head 7 / peer
        # head 15) and the relu+store remain.
        CIS_A = (0, 1, 4, 5)
        hosts = []

        def host_partA(ot, pool, tag):
            if pool is flex_pool:
                h0 = pool.tile([P, 512], f32, tag=tag, name=f"tf{ot}a")
                h1 = pool.tile([P, 512], f32, tag=tag, name=f"tf{ot}b")
                pair = ((h0, 0), (h1, 0))
            else:
                ht = pool.tile([P, THALF], f32, tag=tag, name=f"tf{ot}")
                pair = ((ht, 0), (ht, 512))
            for tc, (ht, c0) in enumerate(pair):
                ffn_mms(ht[:, c0:c0 + 512], 1, ot, tc, CIS_A, True, False)
                hosts.append((ot, tc, ht, c0))

        def ffn_half(ht, lo, hi, ot, tc, cis, stop):
            for k, ci in enumerate(cis):
                nc.tensor.matmul(
                    ht, lhsT=wfT_sb[lo:hi, ci, P * ot:P * (ot + 1)],
                    rhs=ccout_sb[lo:hi, 1, ci, 512 * tc:512 * (tc + 1)],
                    start=False, stop=(stop and k == len(cis) - 1))

        # flex hosts first (PE work while the DVE recip chain of unit (7,1)
        # completes), then the norm tail (zbp must take its sc slot BEFORE
        # the sc hosts, else the slot rotation deadlocks), then sc/av hosts
        host_partA(0, flex_pool, "flex")
        flush_norm()
        host_partA(1, sc_pool, "sc")
        host_partA(2, sc_pool, "sc")
        host_partA(3, av_pool, "av")
        # head 7 of th1: the only tail-exposed collective (128KB)
        allgather(1, 448, 512, "th1d")
        out_c = ag_state["th1c"].rearrange("(k p) t -> k p t", p=64)
        nc.sync.dma_start(out=ccout_sb[0:64, 1, 3, :], in_=out_c[0])
        nc.sync.dma_start(out=ccout_sb[0:64, 1, 7, :], in_=out_c[1])
        # AllGather-latency fill: (2,6) fulls, then ci3/ci7 top halves
        for ot, tc, ht, c0 in hosts:
            ffn_mms(ht[:, c0:c0 + 512], 1, ot, tc, (2, 6), False, False)
        for ot, tc, ht, c0 in hosts:
            ffn_half(ht[:, c0:c0 + 512], 0, 64, ot, tc, (3, 7), False)
        out_d = ag_state["th1d"].rearrange("(k p) t -> k p t", p=64)
        nc.scalar.dma_start(out=ccout_sb[64:P, 1, 3, :], in_=out_d[0])
        nc.scalar.dma_start(out=ccout_sb[64:P, 1, 7, :], in_=out_d[1])
        for ot, tc, ht, c0 in hosts:
            ffn_half(ht[:, c0:c0 + 512], 64, P, ot, tc, (3, 7), True)
            ffn_out(ht[:, c0:c0 + 512], 1, ot, tc)

    nc.compile()
    return nc


def make_in_maps(x, Wq, Wk, Wv, Wf, bf):
    x = np.asarray(x, np.float32)
    mku_m = np.ascontiguousarray(
        np.triu(np.ones((P, P), np.float32))).astype(bf16)
    tri = (np.triu(np.ones((P, P), np.float32)) - 1.0) * MASK_BIG
    mkb_m = np.ascontiguousarray(np.concatenate(
        [np.full((P, P), -MASK_BIG, np.float32), tri], axis=1)).astype(bf16)
    bf_f = np.asarray(bf, np.float32)
    wfT_f = np.asarray(Wf, np.float32).T
    xw_dt = f8e4 if FP8_QKV else bf16
    ws = W_SCALE if FP8_QKV else 1.0
    in_maps = []
    for core in range(8):
        b, p = divmod(core, 2)
        sl = slice(HPC * p, HPC * (p + 1))
        in_maps.append({
            "xT": np.ascontiguousarray(x[b].T).astype(xw_dt),
            "wq": np.ascontiguousarray(
                np.asarray(Wq, np.float32)[:, sl].reshape(C, HPC * D)
                * ws).astype(xw_dt),
            "wk": np.ascontiguousarray(
                np.asarray(Wk, np.float32)[:, sl].reshape(C, HPC * D)
                * ws).astype(xw_dt),
            "wv": np.ascontiguousarray(
                np.asarray(Wv, np.float32)[:, sl].reshape(C, HPC * D)
                * ws).astype(xw_dt),
            "wfT": np.ascontiguousarray(
                wfT_f[:, COH * p:COH * (p + 1)]).astype(bf16),
            "mku": mku_m,
            "mkb": mkb_m,
            "biasco": np.ascontiguousarray(
                bf_f[COH * p:COH * (p + 1)].reshape(NOT, P).T),
        })
    return in_maps


def run(x, Wq, Wk, Wv, Wf, bf, trace=False, **spmd_kwargs):
    from concourse.bass_utils import run_bass_kernel_spmd

    if "nc" not in _CACHE:
        _CACHE["nc"] = build_nc()
    nc = _CACHE["nc"]
    in_maps = make_in_maps(x, Wq, Wk, Wv, Wf, bf)
    res = run_bass_kernel_spmd(
        nc, in_maps, core_ids=list(range(8)), trace=trace, **spmd_kwargs)
    out = np.zeros((B, T, C), np.float32)
    for core in range(8):
        b, p = divmod(core, 2)
        out[b, :, COH * p:COH * (p + 1)] = \
            res.results[core]["y"].T.astype(np.float32)
    return out, res


def kernel(x, Wq, Wk, Wv, Wf, bf):
    try:
        out, _ = run(x, Wq, Wk, Wv, Wf, bf, trace=False)
    except Exception:
        # transient device flake (hang/unrecoverable): one retry
        import time
        time.sleep(2.0)
        out, _ = run(x, Wq, Wk, Wv, Wf, bf, trace=False)
    return out

